# revision 6
# baseline (speedup 1.0000x reference)
"""Trainium2 Bass kernel for the Backflow module.

Math (B=16, N=512, DIM=3, H=32):
  out[b,i,:] = sum_j eta(||x_bi - x_bj||) * (x_bi - x_bj)  +  mu(||x_bi||) * x_bi
where eta/mu are 1->H->1 tanh MLPs. The reference's eye()/diagonal correction
cancels exactly: with d_ii = 0 the diagonal term is eta(0)*0 = 0, which equals
the reference's (include-diagonal then subtract eta(sqrt(DIM))) formulation.

Sharding: data-parallel over batch, 2 batches per core on 8 cores; the tiny
MLP parameters are replicated.

Per-core layout: i on partitions (4 chunks of 128), j on the free dim.
  - dx_c[i,j] = x[j,c] - x[i,c] via tensor_scalar (sign absorbed by negating
    eta_w2/eta_b2), d2 = sum_c dx_c^2, d = sqrt(d2) on ACT.
  - eta tanh: 32 ACT instructions over [128, 2048] (all 4 chunks at once),
    second layer as PE matmuls with stationary diag(-w2_k) accumulating in
    PSUM.
  - row-sums sum_j (-eta)*dx_c fused into one scalar_tensor_tensor per (I,c)
    via accum_out.
  - ACT table sets: sqrt and tanh never share a set, so all Sqrt work of a
    batch is grouped before all Tanh work (2 table loads per batch).
"""

import sys

sys.path.insert(0, "/opt/trn_rl_repo")

import numpy as np
from contextlib import ExitStack

B, N, DIM, H = 16, 512, 3, 32
NCORES = 8
BPC = B // NCORES  # batches per core
P = 128
NCHUNK = N // P  # 4
NQ = NCHUNK * N  # 2048: quad free dim

LAST_RESULT = None


def _spread_sync_waits(nc):
    """The pinned walrus rejects instructions carrying more than one sync wait
    ('Too many sync wait commands'). Engines execute their instruction streams
    in order, so hoist all-but-one wait of any such instruction onto same-engine
    NoOps inserted directly before it — semantically identical ordering."""
    from concourse import mybir

    n_added = 0
    for bb in nc.main_func.blocks:
        insts = bb.instructions
        i = 0
        while i < len(insts):
            inst = insts[i]
            si = getattr(inst, "sync_info", None)
            waits = list(si.on_wait) if si is not None and si.on_wait else []
            if len(waits) > 1:
                si.on_wait = waits[-1:]
                for k, w in enumerate(waits[:-1]):
                    nop = mybir.InstNoOp(
                        name=f"{inst.name}-wspread{k}",
                        sync_info=mybir.SyncInfo(on_wait=[w], on_update=[]),
                        engine=inst.engine,
                        bass_nofuse=True,
                    )
                    insts.insert(i + k, nop)
                    n_added += 1
                i += len(waits) - 1
            i += 1
    return n_added


def _build_program(neg_eta_b2: float, mu_b2_val: float):
    import concourse.bass as bass
    import concourse.tile as tile
    from concourse import mybir

    f32 = mybir.dt.float32
    AF = mybir.ActivationFunctionType
    OP = mybir.AluOpType
    AX = mybir.AxisListType

    nc = bass.Bass()
    x_d = nc.dram_tensor("x", [BPC, N, DIM], f32, kind="ExternalInput")
    xT_d = nc.dram_tensor("xT", [BPC, DIM, N], f32, kind="ExternalInput")
    w2diag_d = nc.dram_tensor("w2diag", [H, P, P], f32, kind="ExternalInput")
    etas_d = nc.dram_tensor("etas", [P, 2, H], f32, kind="ExternalInput")
    mus_d = nc.dram_tensor("mus", [H, 2], f32, kind="ExternalInput")
    muw2_d = nc.dram_tensor("muw2", [H, DIM], f32, kind="ExternalInput")
    ident_d = nc.dram_tensor("ident", [P, P], f32, kind="ExternalInput")
    out_d = nc.dram_tensor("out", [BPC, N, DIM], f32, kind="ExternalOutput")

    with tile.TileContext(nc) as tc, ExitStack() as ctx:
        singles = ctx.enter_context(tc.tile_pool(name="singles", bufs=1))
        dxp = ctx.enter_context(tc.tile_pool(name="dxp", bufs=2))
        sqp = ctx.enter_context(tc.tile_pool(name="sqp", bufs=1))
        d2p = ctx.enter_context(tc.tile_pool(name="d2p", bufs=2))
        dqp = ctx.enter_context(tc.tile_pool(name="dqp", bufs=2))
        hp = ctx.enter_context(tc.tile_pool(name="hp", bufs=4))
        scr = ctx.enter_context(tc.tile_pool(name="scr", bufs=2))
        eep = ctx.enter_context(tc.tile_pool(name="eep", bufs=4))
        enp = ctx.enter_context(tc.tile_pool(name="enp", bufs=2))
        orp = ctx.enter_context(tc.tile_pool(name="orp", bufs=2))
        psacc = ctx.enter_context(tc.tile_pool(name="psacc", bufs=1, space="PSUM"))
        psen = ctx.enter_context(tc.tile_pool(name="psen", bufs=2, space="PSUM"))
        pset = ctx.enter_context(tc.tile_pool(name="pset", bufs=2, space="PSUM"))

        # ---- constants / replicated inputs ----
        w2diag_sb = singles.tile([P, H, P], f32)
        nc.gpsimd.dma_start(
            out=w2diag_sb[:], in_=w2diag_d[:].rearrange("k p m -> p k m")
        )
        etas_sb = singles.tile([P, 2, H], f32)
        nc.gpsimd.dma_start(out=etas_sb[:], in_=etas_d[:])
        mus_sb = singles.tile([H, 2], f32)
        nc.gpsimd.dma_start(out=mus_sb[:], in_=mus_d[:])
        muw2_sb = singles.tile([H, DIM], f32)
        nc.gpsimd.dma_start(out=muw2_sb[:], in_=muw2_d[:])
        ident_sb = singles.tile([P, P], f32)
        nc.gpsimd.dma_start(out=ident_sb[:], in_=ident_d[:])

        xi_sb = singles.tile([P, BPC, NCHUNK, DIM], f32)
        nc.gpsimd.dma_start(
            out=xi_sb[:], in_=x_d[:].rearrange("b (i p) c -> p b i c", p=P)
        )
        xT_sb = singles.tile([DIM, BPC, N], f32)
        nc.gpsimd.dma_start(out=xT_sb[:], in_=xT_d[:].rearrange("b c j -> c b j"))
        # x[j,c] replicated across all 128 partitions: partition-stride-0 DMA
        xjrep_sb = singles.tile([P, BPC, DIM, N], f32)
        bcast_src = bass.AP(
            tensor=xT_d[:].tensor,
            offset=0,
            ap=[[0, P], [DIM * N, BPC], [N, DIM], [1, N]],
        )
        nc.gpsimd.dma_start(out=xjrep_sb[:], in_=bcast_src)

        ones3 = singles.tile([DIM, 1], f32)
        nc.vector.memset(ones3[:], 1.0)
        ones1_32 = singles.tile([1, H], f32)
        nc.vector.memset(ones1_32[:], 1.0)

        for b in range(BPC):
            # ---- pair-grid prep on DVE (quad layout: free dim = 4*512) ----
            dxq = dxp.tile([P, DIM, NQ], f32)
            for I in range(NCHUNK):
                for c in range(DIM):
                    nc.vector.tensor_scalar(
                        out=dxq[:, c, I * N : (I + 1) * N],
                        in0=xjrep_sb[:, b, c, :],
                        scalar1=xi_sb[:, b, I, c : c + 1],
                        scalar2=None,
                        op0=OP.subtract,
                    )
            sqq = sqp.tile([P, DIM, NQ], f32)
            nc.vector.tensor_mul(sqq[:], dxq[:], dxq[:])
            d2q = d2p.tile([P, NQ], f32)
            nc.vector.reduce_sum(
                out=d2q[:], in_=sqq[:].rearrange("p c q -> p q c"), axis=AX.X
            )
            # ---- sqrt phase (ACT, sqrt table set) ----
            dq = dqp.tile([P, NQ], f32)
            nc.scalar.activation(dq[:], d2q[:], AF.Sqrt)
            # e_n: di = ||x_i|| as a [1, N] row
            sq3 = enp.tile([DIM, N], f32)
            nc.vector.tensor_mul(sq3[:], xT_sb[:, b, :], xT_sb[:, b, :])
            ni_ps = psen.tile([1, N], f32, tag="en")
            nc.tensor.matmul(ni_ps[:], ones3[:], sq3[:], start=True, stop=True)
            di = enp.tile([1, N], f32)
            nc.scalar.activation(di[:], ni_ps[:], AF.Sqrt)

            # ---- tanh phase (ACT, tanh table set) ----
            acc = psacc.tile([P, NCHUNK, N], f32)
            for k in range(H):
                hq = hp.tile([P, NQ], f32)
                nc.scalar.activation(
                    hq[:],
                    dq[:],
                    AF.Tanh,
                    scale=etas_sb[:, 0, k : k + 1],
                    bias=etas_sb[:, 1, k : k + 1],
                )
                for I in range(NCHUNK):
                    nc.tensor.matmul(
                        acc[:, I, :],
                        w2diag_sb[:, k, :],
                        hq[:, I * N : (I + 1) * N],
                        start=(k == 0),
                        stop=(k == H - 1),
                    )
            # mu hidden layer on 32 partitions: tanh(w1*di + b1)
            direp_ps = psen.tile([H, N], f32, tag="en")
            nc.tensor.matmul(direp_ps[:], ones1_32[:], di[:], start=True, stop=True)
            hmu = enp.tile([H, N], f32)
            nc.scalar.activation(
                hmu[:],
                direp_ps[:],
                AF.Tanh,
                scale=mus_sb[:, 0:1],
                bias=mus_sb[:, 1:2],
            )
            mu_ps = psen.tile([DIM, N], f32, tag="en")
            nc.tensor.matmul(mu_ps[:], muw2_sb[:], hmu[:], start=True, stop=True)
            en = enp.tile([DIM, N], f32)
            nc.vector.scalar_tensor_tensor(
                out=en[:],
                in0=mu_ps[:],
                scalar=mu_b2_val,
                in1=xT_sb[:, b, :],
                op0=OP.add,
                op1=OP.mult,
            )

            # ---- finalize: e_e row sums, transpose to [c, i], add e_n ----
            outrow = orp.tile([DIM, N], f32)
            for I in range(NCHUNK):
                ee = eep.tile([P, DIM], f32)
                for c in range(DIM):
                    scratch = scr.tile([P, N], f32)
                    nc.vector.scalar_tensor_tensor(
                        out=scratch[:],
                        in0=acc[:, I, :],
                        scalar=neg_eta_b2,
                        in1=dxq[:, c, I * N : (I + 1) * N],
                        op0=OP.add,
                        op1=OP.mult,
                        accum_out=ee[:, c : c + 1],
                    )
                eeT_ps = pset.tile([DIM, P], f32)
                nc.tensor.transpose(eeT_ps[:], ee[:], ident_sb[:])
                nc.vector.tensor_add(
                    outrow[:, I * P : (I + 1) * P],
                    eeT_ps[:],
                    en[:, I * P : (I + 1) * P],
                )
            nc.gpsimd.dma_start(
                out=out_d[b].rearrange("i c -> c i"), in_=outrow[:]
            )

    _spread_sync_waits(nc)
    return nc


def _ensure_ntff_hook():
    """bass_utils' axon trace path imports antenv.axon_hooks, which the image's
    antenv package lacks. Register an equivalent module backed by the boot
    package's ctypes NTFF hook so trace=True works; degrade silently if the
    pieces are missing (tracing is optional)."""
    import os
    import types

    try:
        import antenv.axon_hooks  # noqa: F401

        return
    except ImportError:
        pass
    try:
        import antenv
    except ImportError:
        return
    mod = types.ModuleType("antenv.axon_hooks")
    box = {"h": None}
    mod.set_axon_ntff_profile_hook = lambda h: box.__setitem__("h", h)
    mod.get_axon_ntff_profile_hook = lambda: box["h"]
    sys.modules["antenv.axon_hooks"] = mod
    antenv.axon_hooks = mod
    try:
        from trn_agent_boot.trn_boot import _ntff_profile_via_ctypes

        so = "/opt/axon/libaxon_pjrt.so"
        if os.path.exists(so):
            hook = _ntff_profile_via_ctypes(so)
            if hook is not None:
                mod.set_axon_ntff_profile_hook(hook)
    except Exception:
        pass


def kernel(x, eta_w1, eta_b1, eta_w2, eta_b2, mu_w1, mu_b1, mu_w2, mu_b2):
    global LAST_RESULT
    _ensure_ntff_hook()
    from concourse.bass_utils import run_bass_kernel_spmd

    f32 = np.float32
    x = np.ascontiguousarray(np.asarray(x, dtype=f32))
    eta_w1 = np.asarray(eta_w1, f32)
    eta_b1 = np.asarray(eta_b1, f32)
    eta_w2 = np.asarray(eta_w2, f32)
    eta_b2 = np.asarray(eta_b2, f32)
    mu_w1 = np.asarray(mu_w1, f32)
    mu_b1 = np.asarray(mu_b1, f32)
    mu_w2 = np.asarray(mu_w2, f32)
    mu_b2 = np.asarray(mu_b2, f32)

    nc = _build_program(float(-eta_b2[0]), float(mu_b2[0]))

    w2diag = np.zeros((H, P, P), f32)
    idx = np.arange(P)
    w2diag[:, idx, idx] = -eta_w2[:, 0][:, None]
    etas = np.zeros((P, 2, H), f32)
    etas[:, 0, :] = eta_w1[0][None, :]
    etas[:, 1, :] = eta_b1[None, :]
    mus = np.stack([mu_w1[0], mu_b1], axis=1).astype(f32)  # [H, 2]
    muw2 = np.repeat(mu_w2, DIM, axis=1).astype(f32)  # [H, DIM]
    ident = np.eye(P, dtype=f32)

    in_maps = []
    for core in range(NCORES):
        xc = np.ascontiguousarray(x[core * BPC : (core + 1) * BPC])
        xTc = np.ascontiguousarray(xc.transpose(0, 2, 1))
        in_maps.append(
            {
                "x": xc,
                "xT": xTc,
                "w2diag": w2diag,
                "etas": etas,
                "mus": mus,
                "muw2": muw2,
                "ident": ident,
            }
        )

    res = run_bass_kernel_spmd(nc, in_maps, core_ids=list(range(NCORES)))
    LAST_RESULT = res
    out = np.concatenate([r["out"] for r in res.results], axis=0)
    return out.astype(np.float32)


# revision 8
# speedup vs baseline: 1.4888x; 1.4888x over previous
"""Trainium2 Bass kernel for the Backflow module.

Math (B=16, N=512, DIM=3, H=32):
  out[b,i,:] = sum_j eta(||x_bi - x_bj||) * (x_bi - x_bj)  +  mu(||x_bi||) * x_bi
where eta/mu are 1->H->1 tanh MLPs. The reference's eye()/diagonal correction
cancels exactly: with d_ii = 0 the diagonal term is eta(0)*0 = 0, which equals
the reference's (include-diagonal then subtract eta(sqrt(DIM))) formulation.

Sharding: data-parallel over batch, 2 batches per core on 8 cores; the tiny
MLP parameters are replicated.

Per-core layout: i on partitions (4 chunks of 128), j on the free dim.
  - dx_c[i,j] = x[j,c] - x[i,c] via tensor_scalar (sign absorbed by negating
    eta_w2/eta_b2), d2 = sum_c dx_c^2, d = sqrt(d2) on ACT.
  - eta tanh: 32 ACT instructions over [128, 2048] (all 4 chunks at once),
    second layer as PE matmuls with stationary diag(-w2_k) accumulating in
    PSUM.
  - row-sums sum_j (-eta)*dx_c fused into one scalar_tensor_tensor per (I,c)
    via accum_out.
  - ACT table sets: sqrt and tanh never share a set, so all Sqrt work of a
    batch is grouped before all Tanh work (2 table loads per batch).
"""

import sys

sys.path.insert(0, "/opt/trn_rl_repo")

import numpy as np
from contextlib import ExitStack

B, N, DIM, H = 16, 512, 3, 32
NCORES = 8
BPC = B // NCORES  # batches per core
P = 128
NCHUNK = N // P  # 4
NQ = NCHUNK * N  # 2048: quad free dim

LAST_RESULT = None


def _spread_sync_waits(nc):
    """The pinned walrus rejects instructions carrying more than one sync wait
    ('Too many sync wait commands'). Engines execute their instruction streams
    in order, so hoist all-but-one wait of any such instruction onto same-engine
    NoOps inserted directly before it — semantically identical ordering."""
    from concourse import mybir

    n_added = 0
    for bb in nc.main_func.blocks:
        insts = bb.instructions
        i = 0
        while i < len(insts):
            inst = insts[i]
            si = getattr(inst, "sync_info", None)
            waits = list(si.on_wait) if si is not None and si.on_wait else []
            if len(waits) > 1:
                si.on_wait = waits[-1:]
                for k, w in enumerate(waits[:-1]):
                    nop = mybir.InstNoOp(
                        name=f"{inst.name}-wspread{k}",
                        sync_info=mybir.SyncInfo(on_wait=[w], on_update=[]),
                        engine=inst.engine,
                        bass_nofuse=True,
                    )
                    insts.insert(i + k, nop)
                    n_added += 1
                i += len(waits) - 1
            i += 1
    return n_added


def _build_program(neg_eta_b2: float, mu_b2_val: float):
    import concourse.bass as bass
    import concourse.tile as tile
    from concourse import mybir

    f32 = mybir.dt.float32
    f32r = mybir.dt.float32r
    AF = mybir.ActivationFunctionType
    OP = mybir.AluOpType
    AX = mybir.AxisListType

    nc = bass.Bass()
    x_d = nc.dram_tensor("x", [BPC, N, DIM], f32, kind="ExternalInput")
    xT_d = nc.dram_tensor("xT", [BPC, DIM, N], f32, kind="ExternalInput")
    w2diag_d = nc.dram_tensor("w2diag", [H, P, P], f32, kind="ExternalInput")
    etas_d = nc.dram_tensor("etas", [P, 2, H], f32, kind="ExternalInput")
    mus_d = nc.dram_tensor("mus", [H, 2], f32, kind="ExternalInput")
    muw2_d = nc.dram_tensor("muw2", [H, DIM], f32, kind="ExternalInput")
    ident_d = nc.dram_tensor("ident", [P, P], f32, kind="ExternalInput")
    out_d = nc.dram_tensor("out", [BPC, N, DIM], f32, kind="ExternalOutput")

    with tile.TileContext(nc) as tc, ExitStack() as ctx:
        singles = ctx.enter_context(tc.tile_pool(name="singles", bufs=1))
        dxp = ctx.enter_context(tc.tile_pool(name="dxp", bufs=2))
        sqp = ctx.enter_context(tc.tile_pool(name="sqp", bufs=1))
        d2p = ctx.enter_context(tc.tile_pool(name="d2p", bufs=2))
        dqp = ctx.enter_context(tc.tile_pool(name="dqp", bufs=2))
        hp = ctx.enter_context(tc.tile_pool(name="hp", bufs=4))
        scr = ctx.enter_context(tc.tile_pool(name="scr", bufs=2))
        eep = ctx.enter_context(tc.tile_pool(name="eep", bufs=4))
        enp = ctx.enter_context(tc.tile_pool(name="enp", bufs=2))
        orp = ctx.enter_context(tc.tile_pool(name="orp", bufs=2))
        psacc = ctx.enter_context(tc.tile_pool(name="psacc", bufs=1, space="PSUM"))
        psen = ctx.enter_context(tc.tile_pool(name="psen", bufs=2, space="PSUM"))
        pset = ctx.enter_context(tc.tile_pool(name="pset", bufs=2, space="PSUM"))

        # ---- constants / replicated inputs ----
        w2diag_st = sqp.tile([P, H, P], f32, tag="sqq")
        nc.gpsimd.dma_start(
            out=w2diag_st[:], in_=w2diag_d[:].rearrange("k p m -> p k m")
        )
        w2diag_sb = singles.tile([P, H, P], f32r)
        nc.vector.tensor_copy(w2diag_sb[:], w2diag_st[:])
        etas_sb = singles.tile([P, 2, H], f32)
        nc.gpsimd.dma_start(out=etas_sb[:], in_=etas_d[:])
        mus_sb = singles.tile([H, 2], f32)
        nc.gpsimd.dma_start(out=mus_sb[:], in_=mus_d[:])
        muw2_sb = singles.tile([H, DIM], f32)
        nc.gpsimd.dma_start(out=muw2_sb[:], in_=muw2_d[:])
        ident_sb = singles.tile([P, P], f32)
        nc.gpsimd.dma_start(out=ident_sb[:], in_=ident_d[:])

        xi_sb = singles.tile([P, BPC, NCHUNK, DIM], f32)
        nc.gpsimd.dma_start(
            out=xi_sb[:], in_=x_d[:].rearrange("b (i p) c -> p b i c", p=P)
        )
        xT_sb = singles.tile([DIM, BPC, N], f32)
        nc.gpsimd.dma_start(out=xT_sb[:], in_=xT_d[:].rearrange("b c j -> c b j"))
        # x[j,c] replicated across all 128 partitions: partition-stride-0 DMA
        xjrep_sb = singles.tile([P, BPC, DIM, N], f32)
        bcast_src = bass.AP(
            tensor=xT_d[:].tensor,
            offset=0,
            ap=[[0, P], [DIM * N, BPC], [N, DIM], [1, N]],
        )
        nc.gpsimd.dma_start(out=xjrep_sb[:], in_=bcast_src)

        ones3 = singles.tile([DIM, 1], f32)
        nc.vector.memset(ones3[:], 1.0)
        ones1_32 = singles.tile([1, H], f32)
        nc.vector.memset(ones1_32[:], 1.0)

        for b in range(BPC):
            # ---- pair-grid prep on DVE (quad layout: free dim = 4*512) ----
            dxq = dxp.tile([P, DIM, NQ], f32)
            for I in range(NCHUNK):
                for c in range(DIM):
                    nc.vector.tensor_scalar(
                        out=dxq[:, c, I * N : (I + 1) * N],
                        in0=xjrep_sb[:, b, c, :],
                        scalar1=xi_sb[:, b, I, c : c + 1],
                        scalar2=None,
                        op0=OP.subtract,
                    )
            sqq = sqp.tile([P, DIM, NQ], f32, tag="sqq")
            nc.vector.tensor_mul(sqq[:], dxq[:], dxq[:])
            d2q = d2p.tile([P, NQ], f32)
            nc.vector.reduce_sum(
                out=d2q[:], in_=sqq[:].rearrange("p c q -> p q c"), axis=AX.X
            )
            # ---- sqrt phase (ACT, sqrt table set) ----
            dq = dqp.tile([P, NQ], f32)
            nc.scalar.activation(dq[:], d2q[:], AF.Sqrt)
            # e_n: di = ||x_i|| as a [1, N] row
            sq3 = enp.tile([DIM, N], f32)
            nc.vector.tensor_mul(sq3[:], xT_sb[:, b, :], xT_sb[:, b, :])
            ni_ps = psen.tile([1, N], f32, tag="en")
            nc.tensor.matmul(ni_ps[:], ones3[:], sq3[:], start=True, stop=True)
            di = enp.tile([1, N], f32)
            nc.scalar.activation(di[:], ni_ps[:], AF.Sqrt)

            # ---- tanh phase (ACT, tanh table set) ----
            acc = psacc.tile([P, NCHUNK, N], f32)
            for k in range(H):
                hq = hp.tile([P, NQ], f32r)
                nc.scalar.activation(
                    hq[:],
                    dq[:],
                    AF.Tanh,
                    scale=etas_sb[:, 0, k : k + 1],
                    bias=etas_sb[:, 1, k : k + 1],
                )
                for I in range(NCHUNK):
                    nc.tensor.matmul(
                        acc[:, I, :],
                        w2diag_sb[:, k, :],
                        hq[:, I * N : (I + 1) * N],
                        start=(k == 0),
                        stop=(k == H - 1),
                    )
            # mu hidden layer on 32 partitions: tanh(w1*di + b1)
            direp_ps = psen.tile([H, N], f32, tag="en")
            nc.tensor.matmul(direp_ps[:], ones1_32[:], di[:], start=True, stop=True)
            hmu = enp.tile([H, N], f32)
            nc.scalar.activation(
                hmu[:],
                direp_ps[:],
                AF.Tanh,
                scale=mus_sb[:, 0:1],
                bias=mus_sb[:, 1:2],
            )
            mu_ps = psen.tile([DIM, N], f32, tag="en")
            nc.tensor.matmul(mu_ps[:], muw2_sb[:], hmu[:], start=True, stop=True)
            en = enp.tile([DIM, N], f32)
            nc.vector.scalar_tensor_tensor(
                out=en[:],
                in0=mu_ps[:],
                scalar=mu_b2_val,
                in1=xT_sb[:, b, :],
                op0=OP.add,
                op1=OP.mult,
            )

            # ---- finalize: e_e row sums, transpose to [c, i], add e_n ----
            outrow = orp.tile([DIM, N], f32)
            for I in range(NCHUNK):
                ee = eep.tile([P, DIM], f32)
                for c in range(DIM):
                    scratch = scr.tile([P, N], f32)
                    nc.vector.scalar_tensor_tensor(
                        out=scratch[:],
                        in0=acc[:, I, :],
                        scalar=neg_eta_b2,
                        in1=dxq[:, c, I * N : (I + 1) * N],
                        op0=OP.add,
                        op1=OP.mult,
                        accum_out=ee[:, c : c + 1],
                    )
                eeT_ps = pset.tile([DIM, P], f32)
                nc.tensor.transpose(eeT_ps[:], ee[:], ident_sb[:])
                nc.vector.tensor_add(
                    outrow[:, I * P : (I + 1) * P],
                    eeT_ps[:],
                    en[:, I * P : (I + 1) * P],
                )
            nc.gpsimd.dma_start(
                out=out_d[b].rearrange("i c -> c i"), in_=outrow[:]
            )

    _spread_sync_waits(nc)
    return nc


def _ensure_ntff_hook():
    """bass_utils' axon trace path imports antenv.axon_hooks, which the image's
    antenv package lacks. Register an equivalent module backed by the boot
    package's ctypes NTFF hook so trace=True works; degrade silently if the
    pieces are missing (tracing is optional)."""
    import os
    import types

    try:
        import antenv.axon_hooks  # noqa: F401

        return
    except ImportError:
        pass
    try:
        import antenv
    except ImportError:
        return
    mod = types.ModuleType("antenv.axon_hooks")
    box = {"h": None}
    mod.set_axon_ntff_profile_hook = lambda h: box.__setitem__("h", h)
    mod.get_axon_ntff_profile_hook = lambda: box["h"]
    sys.modules["antenv.axon_hooks"] = mod
    antenv.axon_hooks = mod
    try:
        from trn_agent_boot.trn_boot import _ntff_profile_via_ctypes

        so = "/opt/axon/libaxon_pjrt.so"
        if os.path.exists(so):
            hook = _ntff_profile_via_ctypes(so)
            if hook is not None:
                mod.set_axon_ntff_profile_hook(hook)
    except Exception:
        pass


def kernel(x, eta_w1, eta_b1, eta_w2, eta_b2, mu_w1, mu_b1, mu_w2, mu_b2):
    global LAST_RESULT
    _ensure_ntff_hook()
    from concourse.bass_utils import run_bass_kernel_spmd

    f32 = np.float32
    x = np.ascontiguousarray(np.asarray(x, dtype=f32))
    eta_w1 = np.asarray(eta_w1, f32)
    eta_b1 = np.asarray(eta_b1, f32)
    eta_w2 = np.asarray(eta_w2, f32)
    eta_b2 = np.asarray(eta_b2, f32)
    mu_w1 = np.asarray(mu_w1, f32)
    mu_b1 = np.asarray(mu_b1, f32)
    mu_w2 = np.asarray(mu_w2, f32)
    mu_b2 = np.asarray(mu_b2, f32)

    nc = _build_program(float(-eta_b2[0]), float(mu_b2[0]))

    w2diag = np.zeros((H, P, P), f32)
    idx = np.arange(P)
    w2diag[:, idx, idx] = -eta_w2[:, 0][:, None]
    etas = np.zeros((P, 2, H), f32)
    etas[:, 0, :] = eta_w1[0][None, :]
    etas[:, 1, :] = eta_b1[None, :]
    mus = np.stack([mu_w1[0], mu_b1], axis=1).astype(f32)  # [H, 2]
    muw2 = np.repeat(mu_w2, DIM, axis=1).astype(f32)  # [H, DIM]
    ident = np.eye(P, dtype=f32)

    in_maps = []
    for core in range(NCORES):
        xc = np.ascontiguousarray(x[core * BPC : (core + 1) * BPC])
        xTc = np.ascontiguousarray(xc.transpose(0, 2, 1))
        in_maps.append(
            {
                "x": xc,
                "xT": xTc,
                "w2diag": w2diag,
                "etas": etas,
                "mus": mus,
                "muw2": muw2,
                "ident": ident,
            }
        )

    res = run_bass_kernel_spmd(nc, in_maps, core_ids=list(range(NCORES)))
    LAST_RESULT = res
    out = np.concatenate([r["out"] for r in res.results], axis=0)
    return out.astype(np.float32)


# revision 14
# speedup vs baseline: 1.7087x; 1.1477x over previous
"""Trainium2 Bass kernel for the Backflow module.

Math (B=16, N=512, DIM=3, H=32):
  out[b,i,:] = sum_j eta(||x_bi - x_bj||) * (x_bi - x_bj)  +  mu(||x_bi||) * x_bi
where eta/mu are 1->H->1 tanh MLPs. The reference's eye()/diagonal correction
cancels exactly: the matrix form below includes the diagonal in both sums, and
eta(0)*(x_i - x_i) = 0.

Sharding: data-parallel over batch, 2 batches per core on 8 cores; the tiny
MLP parameters are replicated.

Per-core layout: i on partitions (4 chunks of 128), j on the free dim.
Symmetry eta(d_ij) = eta(d_ji): compute only block-triangular strips
(chunk I covers j in [128*I, 512)), packed to [128, 1280] (-37% tanh work).

  M[i,j] := -eta(d_ij) is built in PSUM: 32 tanh ACT ops over the packed strip
  (scale/bias = eta w1/b1 per k), each scaled by -w2_k via a PE matmul with
  stationary diag(-w2_k), plus a ones-matmul adding -b2. float32r is used on
  the matmul path (4x faster than fp32 at moving >= 256; ~tf32 precision).

  Row sums come from PE contractions with stationary [x_I | 1]:
    P_c[m] = sum_n M[m,n] x_c[n],  Q[m] = sum_n M[m,n]
    e_e_c[m] = sum_n eta*(x_c[m]-x_c[n]) = P_c[m] - x_c[m]*Q[m]
  Direct blocks give the (J,*) rows, PE-transposed blocks give the reflected
  (I,*) rows.

  ACT table sets: sqrt and tanh never share a set, so all Sqrt work of a batch
  is grouped before all Tanh work (2 table loads per batch).
"""

import sys

sys.path.insert(0, "/opt/trn_rl_repo")

import numpy as np
from contextlib import ExitStack

B, N, DIM, H = 16, 512, 3, 32
NCORES = 8
BPC = B // NCORES  # batches per core
P = 128
NCHUNK = N // P  # 4
# block-triangular strips: chunk I covers j in [128*I, N)
WIDTHS = [N - P * I for I in range(NCHUNK)]  # [512, 384, 256, 128]
OFFS = [0]
for w in WIDTHS[:-1]:
    OFFS.append(OFFS[-1] + w)
NPACK = sum(WIDTHS)  # 1280
# matmul column splits over the packed strip (N<=512, each >=256 for f32r)
MM_SPLITS = [(0, 512), (512, 512), (1024, 256)]

LAST_RESULT = None


def _spread_sync_waits(nc):
    """The pinned walrus rejects instructions carrying more than one sync wait
    ('Too many sync wait commands'). Engines execute their instruction streams
    in order, so hoist all-but-one wait of any such instruction onto same-engine
    NoOps inserted directly before it — semantically identical ordering."""
    from concourse import mybir

    n_added = 0
    for bb in nc.main_func.blocks:
        insts = bb.instructions
        i = 0
        while i < len(insts):
            inst = insts[i]
            si = getattr(inst, "sync_info", None)
            waits = list(si.on_wait) if si is not None and si.on_wait else []
            if len(waits) > 1:
                si.on_wait = waits[-1:]
                for k, w in enumerate(waits[:-1]):
                    nop = mybir.InstNoOp(
                        name=f"{inst.name}-wspread{k}",
                        sync_info=mybir.SyncInfo(on_wait=[w], on_update=[]),
                        engine=inst.engine,
                        bass_nofuse=True,
                    )
                    insts.insert(i + k, nop)
                    n_added += 1
                i += len(waits) - 1
            i += 1
    return n_added


def _build_program(neg_eta_b2: float, mu_b2_val: float, debug_out: bool = False):
    import concourse.bass as bass
    import concourse.tile as tile
    from concourse import mybir

    f32 = mybir.dt.float32
    f32r = mybir.dt.float32r
    AF = mybir.ActivationFunctionType
    OP = mybir.AluOpType
    AX = mybir.AxisListType

    nc = bass.Bass()
    x_d = nc.dram_tensor("x", [BPC, N, DIM], f32, kind="ExternalInput")
    xT_d = nc.dram_tensor("xT", [BPC, DIM, N], f32, kind="ExternalInput")
    w2diag_d = nc.dram_tensor("w2diag", [H, P, P], f32, kind="ExternalInput")
    etas_d = nc.dram_tensor("etas", [P, 2, H], f32, kind="ExternalInput")
    mus_d = nc.dram_tensor("mus", [H, 2], f32, kind="ExternalInput")
    muw2_d = nc.dram_tensor("muw2", [H, DIM], f32, kind="ExternalInput")
    ident_d = nc.dram_tensor("ident", [P, P], f32, kind="ExternalInput")
    out_d = nc.dram_tensor("out", [BPC, N, DIM], f32, kind="ExternalOutput")
    if debug_out:
        dbg_acc_d = nc.dram_tensor("dbg_acc", [P, NPACK], f32, kind="ExternalOutput")
        dbg_pp_d = nc.dram_tensor("dbg_pp", [DIM, NCHUNK, P], f32, kind="ExternalOutput")
        dbg_pq_d = nc.dram_tensor("dbg_pq", [DIM, NCHUNK, P], f32, kind="ExternalOutput")
        dbg_at_d = nc.dram_tensor("dbg_at", [P, P], f32, kind="ExternalOutput")

    with tile.TileContext(nc) as tc, ExitStack() as ctx:
        singles = ctx.enter_context(tc.tile_pool(name="singles", bufs=1))
        dxp = ctx.enter_context(tc.tile_pool(name="dxp", bufs=2))
        sqp = ctx.enter_context(tc.tile_pool(name="sqp", bufs=1))
        d2p = ctx.enter_context(tc.tile_pool(name="d2p", bufs=2))
        dqp = ctx.enter_context(tc.tile_pool(name="dqp", bufs=2))
        hp = ctx.enter_context(tc.tile_pool(name="hp", bufs=4))
        accsbp = ctx.enter_context(tc.tile_pool(name="accsbp", bufs=2))
        atp = ctx.enter_context(tc.tile_pool(name="atp", bufs=3))
        enp = ctx.enter_context(tc.tile_pool(name="enp", bufs=2))
        orp = ctx.enter_context(tc.tile_pool(name="orp", bufs=2))
        psacc = ctx.enter_context(tc.tile_pool(name="psacc", bufs=1, space="PSUM"))
        psout = ctx.enter_context(tc.tile_pool(name="psout", bufs=1, space="PSUM"))
        pstr = ctx.enter_context(tc.tile_pool(name="pstr", bufs=1, space="PSUM"))
        psen = ctx.enter_context(tc.tile_pool(name="psen", bufs=1, space="PSUM"))

        # ---- constants / replicated inputs ----
        w2diag_st = sqp.tile([P, H, P], f32, tag="sqq")
        nc.gpsimd.dma_start(
            out=w2diag_st[:], in_=w2diag_d[:].rearrange("k p m -> p k m")
        )
        w2diag_sb = singles.tile([P, H, P], f32r)
        nc.vector.tensor_copy(w2diag_sb[:], w2diag_st[:])
        etas_sb = singles.tile([P, 2, H], f32)
        nc.gpsimd.dma_start(out=etas_sb[:], in_=etas_d[:])
        mus_sb = singles.tile([H, 2], f32)
        nc.gpsimd.dma_start(out=mus_sb[:], in_=mus_d[:])
        muw2_sb = singles.tile([H, DIM], f32)
        nc.gpsimd.dma_start(out=muw2_sb[:], in_=muw2_d[:])
        ident_sb = singles.tile([P, P], f32)
        nc.gpsimd.dma_start(out=ident_sb[:], in_=ident_d[:])

        xi_sb = singles.tile([P, BPC, NCHUNK, DIM], f32)
        nc.gpsimd.dma_start(
            out=xi_sb[:], in_=x_d[:].rearrange("b (i p) c -> p b i c", p=P)
        )
        # reflection stationaries: [x_I cols | ones cols] per (b, I)
        statx = singles.tile([P, BPC, NCHUNK, 2 * DIM], f32)
        nc.gpsimd.dma_start(
            out=statx[:, :, :, 0:DIM],
            in_=x_d[:].rearrange("b (i p) c -> p b i c", p=P),
        )
        nc.vector.memset(statx[:, :, :, DIM : 2 * DIM], 1.0)
        xT_sb = singles.tile([DIM, BPC, N], f32)
        nc.gpsimd.dma_start(out=xT_sb[:], in_=xT_d[:].rearrange("b c j -> c b j"))
        # x[j,c] replicated across all 128 partitions: partition-stride-0 DMA
        xjrep_sb = singles.tile([P, BPC, DIM, N], f32)
        bcast_src = bass.AP(
            tensor=xT_d[:].tensor,
            offset=0,
            ap=[[0, P], [DIM * N, BPC], [N, DIM], [1, N]],
        )
        nc.gpsimd.dma_start(out=xjrep_sb[:], in_=bcast_src)

        ones3 = singles.tile([DIM, 1], f32)
        nc.vector.memset(ones3[:], 1.0)
        ones1_32 = singles.tile([1, H], f32)
        nc.vector.memset(ones1_32[:], 1.0)
        onesrow = singles.tile([1, NPACK], f32)
        nc.vector.memset(onesrow[:], 1.0)
        negb2row = singles.tile([1, P], f32)
        nc.vector.memset(negb2row[:], neg_eta_b2)

        for b in range(BPC):
            # ---- pair-grid prep on DVE (packed strips, free dim = 1280) ----
            dxs = dxp.tile([P, DIM, NPACK], f32)
            for I in range(NCHUNK):
                for c in range(DIM):
                    nc.vector.tensor_scalar(
                        out=dxs[:, c, OFFS[I] : OFFS[I] + WIDTHS[I]],
                        in0=xjrep_sb[:, b, c, P * I : N],
                        scalar1=xi_sb[:, b, I, c : c + 1],
                        scalar2=None,
                        op0=OP.subtract,
                    )
            sqs = sqp.tile([P, DIM, NPACK], f32, tag="sqq")
            nc.vector.tensor_mul(sqs[:], dxs[:], dxs[:])
            d2s = d2p.tile([P, NPACK], f32)
            nc.vector.reduce_sum(
                out=d2s[:], in_=sqs[:].rearrange("p c q -> p q c"), axis=AX.X
            )
            # ---- sqrt phase (ACT, sqrt table set) ----
            ds = dqp.tile([P, NPACK], f32)
            nc.scalar.activation(ds[:], d2s[:], AF.Sqrt)
            # e_n: di = ||x_i|| as a [1, N] row
            sq3 = enp.tile([DIM, N], f32)
            nc.vector.tensor_mul(sq3[:], xT_sb[:, b, :], xT_sb[:, b, :])
            ni_ps = psen.tile([1, N], f32, tag="en")
            nc.tensor.matmul(ni_ps[:], ones3[:], sq3[:], start=True, stop=True)
            di = enp.tile([1, N], f32)
            nc.scalar.activation(di[:], ni_ps[:], AF.Sqrt)

            # ---- tanh phase (ACT tanh table set); M = -eta in PSUM ----
            acc = psacc.tile([P, NPACK], f32)
            for k in range(H):
                hs = hp.tile([P, NPACK], f32r)
                nc.scalar.activation(
                    hs[:],
                    ds[:],
                    AF.Tanh,
                    scale=etas_sb[:, 0, k : k + 1],
                    bias=etas_sb[:, 1, k : k + 1],
                )
                for off, w in MM_SPLITS:
                    nc.tensor.matmul(
                        acc[:, off : off + w],
                        w2diag_sb[:, k, :],
                        hs[:, off : off + w],
                        start=(k == 0),
                        stop=False,
                    )
            # -b2 into every entry: stationary -b2 row, moving all-ones row
            for off, w in MM_SPLITS:
                nc.tensor.matmul(
                    acc[:, off : off + w],
                    negb2row[:],
                    onesrow[:, off : off + w],
                    start=False,
                    stop=True,
                )
            # mu hidden layer on 32 partitions: tanh(w1*di + b1)
            direp_ps = psen.tile([H, N], f32, tag="en")
            nc.tensor.matmul(direp_ps[:], ones1_32[:], di[:], start=True, stop=True)
            hmu = enp.tile([H, N], f32)
            nc.scalar.activation(
                hmu[:],
                direp_ps[:],
                AF.Tanh,
                scale=mus_sb[:, 0:1],
                bias=mus_sb[:, 1:2],
            )
            mu_ps = psen.tile([DIM, N], f32, tag="en")
            nc.tensor.matmul(mu_ps[:], muw2_sb[:], hmu[:], start=True, stop=True)
            en = enp.tile([DIM, N], f32)
            nc.vector.scalar_tensor_tensor(
                out=en[:],
                in0=mu_ps[:],
                scalar=mu_b2_val,
                in1=xT_sb[:, b, :],
                op0=OP.add,
                op1=OP.mult,
            )

            # ---- reflection: P_c and Q rows via PE contractions ----
            acc_sb = accsbp.tile([P, NPACK], f32)
            nc.vector.tensor_copy(acc_sb[:], acc[:])

            def blk(I, J):
                off = OFFS[I] + (J - I) * P
                return acc_sb[:, off : off + P]

            # psum_out rows: x_c-weighted sums P_c and plain sums Q, separate
            # tiles because engine APs must start at partition 0
            poutP = psout.tile([DIM, NCHUNK, P], f32, tag="poutP")
            poutQ = psout.tile([DIM, NCHUNK, P], f32, tag="poutQ")
            # start=True resets PSUM state at bank granularity, so exactly one
            # start (the first matmul into each tile) and one stop (the last);
            # per-element has_written bits make later first-touches overwrite
            # and repeat-touches accumulate.
            ncontrib = [0]
            NTOT = NCHUNK * NCHUNK  # 16 contributions per tile

            def contrib(row_chunk, stat_chunk, mov_ap):
                g = ncontrib[0]
                ncontrib[0] += 1
                nc.tensor.matmul(
                    poutP[:, row_chunk, :],
                    statx[:, b, stat_chunk, 0:DIM],
                    mov_ap,
                    start=(g == 0),
                    stop=(g == NTOT - 1),
                    skip_group_check=True,
                )
                nc.tensor.matmul(
                    poutQ[:, row_chunk, :],
                    statx[:, b, stat_chunk, DIM : 2 * DIM],
                    mov_ap,
                    start=(g == 0),
                    stop=(g == NTOT - 1),
                    skip_group_check=True,
                )

            # diagonal blocks first (start=True for each row-chunk)
            for I in range(NCHUNK):
                contrib(I, I, blk(I, I))
            # off-diagonal: direct gives rows J; transposed gives rows I
            for I in range(NCHUNK):
                for J in range(I + 1, NCHUNK):
                    contrib(J, I, blk(I, J))
            for I in range(NCHUNK):
                for J in range(I + 1, NCHUNK):
                    tps = pstr.tile([P, P], f32)
                    nc.tensor.transpose(tps[:], blk(I, J), ident_sb[:])
                    at_sb = atp.tile([P, P], f32)
                    nc.vector.tensor_copy(at_sb[:], tps[:])
                    if debug_out and b == 0 and I == 0 and J == 1:
                        nc.gpsimd.dma_start(out=dbg_at_d[:], in_=at_sb[:])
                    contrib(I, J, at_sb[:])

            # ---- finalize: e_c = P_c - x_c*Q + e_n, in [c, i] layout ----
            outrow = orp.tile([DIM, N], f32)
            for I in range(NCHUNK):
                xq = enp.tile([DIM, P], f32, tag="xq")
                nc.vector.tensor_mul(
                    xq[:], xT_sb[:, b, I * P : (I + 1) * P], poutQ[:, I, :]
                )
                pm = enp.tile([DIM, P], f32, tag="pm")
                nc.vector.tensor_sub(pm[:], poutP[:, I, :], xq[:])
                nc.vector.tensor_add(
                    outrow[:, I * P : (I + 1) * P],
                    pm[:],
                    en[:, I * P : (I + 1) * P],
                )
            nc.gpsimd.dma_start(
                out=out_d[b].rearrange("i c -> c i"), in_=outrow[:]
            )
            if debug_out and b == 0:
                nc.gpsimd.dma_start(out=dbg_acc_d[:], in_=acc_sb[:])
                ppsb = orp.tile([DIM, NCHUNK, P], f32, tag="dbgpp")
                nc.vector.tensor_copy(ppsb[:], poutP[:])
                nc.gpsimd.dma_start(out=dbg_pp_d[:], in_=ppsb[:])
                pqsb = orp.tile([DIM, NCHUNK, P], f32, tag="dbgpq")
                nc.vector.tensor_copy(pqsb[:], poutQ[:])
                nc.gpsimd.dma_start(out=dbg_pq_d[:], in_=pqsb[:])

    _spread_sync_waits(nc)
    return nc


def _ensure_ntff_hook():
    """bass_utils' axon trace path imports antenv.axon_hooks, which the image's
    antenv package lacks. Register an equivalent module backed by the boot
    package's ctypes NTFF hook so trace=True works; degrade silently if the
    pieces are missing (tracing is optional)."""
    import os
    import types

    try:
        import antenv.axon_hooks  # noqa: F401

        return
    except ImportError:
        pass
    try:
        import antenv
    except ImportError:
        return
    mod = types.ModuleType("antenv.axon_hooks")
    box = {"h": None}
    mod.set_axon_ntff_profile_hook = lambda h: box.__setitem__("h", h)
    mod.get_axon_ntff_profile_hook = lambda: box["h"]
    sys.modules["antenv.axon_hooks"] = mod
    antenv.axon_hooks = mod
    try:
        from trn_agent_boot.trn_boot import _ntff_profile_via_ctypes

        so = "/opt/axon/libaxon_pjrt.so"
        if os.path.exists(so):
            hook = _ntff_profile_via_ctypes(so)
            if hook is not None:
                mod.set_axon_ntff_profile_hook(hook)
    except Exception:
        pass


def kernel(x, eta_w1, eta_b1, eta_w2, eta_b2, mu_w1, mu_b1, mu_w2, mu_b2):
    global LAST_RESULT
    _ensure_ntff_hook()
    from concourse.bass_utils import run_bass_kernel_spmd

    f32 = np.float32
    x = np.ascontiguousarray(np.asarray(x, dtype=f32))
    eta_w1 = np.asarray(eta_w1, f32)
    eta_b1 = np.asarray(eta_b1, f32)
    eta_w2 = np.asarray(eta_w2, f32)
    eta_b2 = np.asarray(eta_b2, f32)
    mu_w1 = np.asarray(mu_w1, f32)
    mu_b1 = np.asarray(mu_b1, f32)
    mu_w2 = np.asarray(mu_w2, f32)
    mu_b2 = np.asarray(mu_b2, f32)

    nc = _build_program(float(-eta_b2[0]), float(mu_b2[0]))

    w2diag = np.zeros((H, P, P), f32)
    idx = np.arange(P)
    w2diag[:, idx, idx] = -eta_w2[:, 0][:, None]
    etas = np.zeros((P, 2, H), f32)
    etas[:, 0, :] = eta_w1[0][None, :]
    etas[:, 1, :] = eta_b1[None, :]
    mus = np.stack([mu_w1[0], mu_b1], axis=1).astype(f32)  # [H, 2]
    muw2 = np.repeat(mu_w2, DIM, axis=1).astype(f32)  # [H, DIM]
    ident = np.eye(P, dtype=f32)

    in_maps = []
    for core in range(NCORES):
        xc = np.ascontiguousarray(x[core * BPC : (core + 1) * BPC])
        xTc = np.ascontiguousarray(xc.transpose(0, 2, 1))
        in_maps.append(
            {
                "x": xc,
                "xT": xTc,
                "w2diag": w2diag,
                "etas": etas,
                "mus": mus,
                "muw2": muw2,
                "ident": ident,
            }
        )

    res = run_bass_kernel_spmd(nc, in_maps, core_ids=list(range(NCORES)))
    LAST_RESULT = res
    out = np.concatenate([r["out"] for r in res.results], axis=0)
    return out.astype(np.float32)


# revision 15
# speedup vs baseline: 1.7217x; 1.0076x over previous
"""Trainium2 Bass kernel for the Backflow module.

Math (B=16, N=512, DIM=3, H=32):
  out[b,i,:] = sum_j eta(||x_bi - x_bj||) * (x_bi - x_bj)  +  mu(||x_bi||) * x_bi
where eta/mu are 1->H->1 tanh MLPs. The reference's eye()/diagonal correction
cancels exactly: the matrix form below includes the diagonal in both sums, and
eta(0)*(x_i - x_i) = 0.

Sharding: data-parallel over batch, 2 batches per core on 8 cores; the tiny
MLP parameters are replicated.

Per-core layout: i on partitions (4 chunks of 128), j on the free dim.
Symmetry eta(d_ij) = eta(d_ji): compute only block-triangular strips
(chunk I covers j in [128*I, 512)), packed to [128, 1280] (-37% tanh work).

  M[i,j] := -eta(d_ij) is built in PSUM: 32 tanh ACT ops over the packed strip
  (scale/bias = eta w1/b1 per k), each scaled by -w2_k via a PE matmul with
  stationary diag(-w2_k), plus a ones-matmul adding -b2. float32r is used on
  the matmul path (4x faster than fp32 at moving >= 256; ~tf32 precision).

  Row sums come from PE contractions with stationary [x_I | 1]:
    P_c[m] = sum_n M[m,n] x_c[n],  Q[m] = sum_n M[m,n]
    e_e_c[m] = sum_n eta*(x_c[m]-x_c[n]) = P_c[m] - x_c[m]*Q[m]
  Direct blocks give the (J,*) rows, PE-transposed blocks give the reflected
  (I,*) rows.

  ACT table sets: sqrt and tanh never share a set, so all Sqrt work of a batch
  is grouped before all Tanh work (2 table loads per batch).
"""

import sys

sys.path.insert(0, "/opt/trn_rl_repo")

import numpy as np
from contextlib import ExitStack

B, N, DIM, H = 16, 512, 3, 32
NCORES = 8
BPC = B // NCORES  # batches per core
P = 128
NCHUNK = N // P  # 4
# block-triangular strips: chunk I covers j in [128*I, N)
WIDTHS = [N - P * I for I in range(NCHUNK)]  # [512, 384, 256, 128]
OFFS = [0]
for w in WIDTHS[:-1]:
    OFFS.append(OFFS[-1] + w)
NPACK = sum(WIDTHS)  # 1280
# matmul column splits over the packed strip (N<=512, each >=256 for f32r)
MM_SPLITS = [(0, 512), (512, 512), (1024, 256)]

LAST_RESULT = None


def _spread_sync_waits(nc):
    """The pinned walrus rejects instructions carrying more than one sync wait
    ('Too many sync wait commands'). Engines execute their instruction streams
    in order, so hoist all-but-one wait of any such instruction onto same-engine
    NoOps inserted directly before it — semantically identical ordering."""
    from concourse import mybir

    n_added = 0
    for bb in nc.main_func.blocks:
        insts = bb.instructions
        i = 0
        while i < len(insts):
            inst = insts[i]
            si = getattr(inst, "sync_info", None)
            waits = list(si.on_wait) if si is not None and si.on_wait else []
            if len(waits) > 1:
                si.on_wait = waits[-1:]
                for k, w in enumerate(waits[:-1]):
                    nop = mybir.InstNoOp(
                        name=f"{inst.name}-wspread{k}",
                        sync_info=mybir.SyncInfo(on_wait=[w], on_update=[]),
                        engine=inst.engine,
                        bass_nofuse=True,
                    )
                    insts.insert(i + k, nop)
                    n_added += 1
                i += len(waits) - 1
            i += 1
    return n_added


def _build_program(neg_eta_b2: float, mu_b2_val: float, debug_out: bool = False):
    import concourse.bass as bass
    import concourse.tile as tile
    from concourse import mybir

    f32 = mybir.dt.float32
    f32r = mybir.dt.float32r
    AF = mybir.ActivationFunctionType
    OP = mybir.AluOpType
    AX = mybir.AxisListType

    nc = bass.Bass()
    x_d = nc.dram_tensor("x", [BPC, N, DIM], f32, kind="ExternalInput")
    xT_d = nc.dram_tensor("xT", [BPC, DIM, N], f32, kind="ExternalInput")
    w2diag_d = nc.dram_tensor("w2diag", [H, P, P], f32, kind="ExternalInput")
    etas_d = nc.dram_tensor("etas", [P, 2, H], f32, kind="ExternalInput")
    mus_d = nc.dram_tensor("mus", [H, 2], f32, kind="ExternalInput")
    muw2_d = nc.dram_tensor("muw2", [H, DIM], f32, kind="ExternalInput")
    ident_d = nc.dram_tensor("ident", [P, P], f32, kind="ExternalInput")
    out_d = nc.dram_tensor("out", [BPC, N, DIM], f32, kind="ExternalOutput")
    if debug_out:
        dbg_acc_d = nc.dram_tensor("dbg_acc", [P, NPACK], f32, kind="ExternalOutput")
        dbg_pp_d = nc.dram_tensor("dbg_pp", [DIM, NCHUNK, P], f32, kind="ExternalOutput")
        dbg_pq_d = nc.dram_tensor("dbg_pq", [DIM, NCHUNK, P], f32, kind="ExternalOutput")
        dbg_at_d = nc.dram_tensor("dbg_at", [P, P], f32, kind="ExternalOutput")

    with tile.TileContext(nc) as tc, ExitStack() as ctx:
        singles = ctx.enter_context(tc.tile_pool(name="singles", bufs=1))
        dxp = ctx.enter_context(tc.tile_pool(name="dxp", bufs=2))
        sqp = ctx.enter_context(tc.tile_pool(name="sqp", bufs=1))
        d2p = ctx.enter_context(tc.tile_pool(name="d2p", bufs=2))
        dqp = ctx.enter_context(tc.tile_pool(name="dqp", bufs=2))
        hp = ctx.enter_context(tc.tile_pool(name="hp", bufs=4))
        accsbp = ctx.enter_context(tc.tile_pool(name="accsbp", bufs=2))
        atp = ctx.enter_context(tc.tile_pool(name="atp", bufs=3))
        enp = ctx.enter_context(tc.tile_pool(name="enp", bufs=2))
        orp = ctx.enter_context(tc.tile_pool(name="orp", bufs=2))
        psacc = ctx.enter_context(tc.tile_pool(name="psacc", bufs=1, space="PSUM"))
        psout = ctx.enter_context(tc.tile_pool(name="psout", bufs=1, space="PSUM"))
        pstr = ctx.enter_context(tc.tile_pool(name="pstr", bufs=1, space="PSUM"))
        psen = ctx.enter_context(tc.tile_pool(name="psen", bufs=1, space="PSUM"))

        # ---- constants / replicated inputs ----
        w2diag_st = sqp.tile([P, H, P], f32, tag="sqq")
        nc.gpsimd.dma_start(
            out=w2diag_st[:], in_=w2diag_d[:].rearrange("k p m -> p k m")
        )
        w2diag_sb = singles.tile([P, H, P], f32r)
        nc.vector.tensor_copy(w2diag_sb[:], w2diag_st[:])
        etas_sb = singles.tile([P, 2, H], f32)
        nc.gpsimd.dma_start(out=etas_sb[:], in_=etas_d[:])
        mus_sb = singles.tile([H, 2], f32)
        nc.gpsimd.dma_start(out=mus_sb[:], in_=mus_d[:])
        muw2_sb = singles.tile([H, DIM], f32)
        nc.gpsimd.dma_start(out=muw2_sb[:], in_=muw2_d[:])
        ident_sb = singles.tile([P, P], f32)
        nc.gpsimd.dma_start(out=ident_sb[:], in_=ident_d[:])

        xi_sb = singles.tile([P, BPC, NCHUNK, DIM], f32)
        nc.gpsimd.dma_start(
            out=xi_sb[:], in_=x_d[:].rearrange("b (i p) c -> p b i c", p=P)
        )
        # reflection stationaries: [x_I cols | ones cols] per (b, I)
        statx = singles.tile([P, BPC, NCHUNK, 2 * DIM], f32)
        nc.gpsimd.dma_start(
            out=statx[:, :, :, 0:DIM],
            in_=x_d[:].rearrange("b (i p) c -> p b i c", p=P),
        )
        nc.vector.memset(statx[:, :, :, DIM : 2 * DIM], 1.0)
        xT_sb = singles.tile([DIM, BPC, N], f32)
        nc.gpsimd.dma_start(out=xT_sb[:], in_=xT_d[:].rearrange("b c j -> c b j"))
        # x[j,c] replicated across all 128 partitions: partition-stride-0 DMA
        xjrep_sb = singles.tile([P, BPC, DIM, N], f32)
        bcast_src = bass.AP(
            tensor=xT_d[:].tensor,
            offset=0,
            ap=[[0, P], [DIM * N, BPC], [N, DIM], [1, N]],
        )
        nc.gpsimd.dma_start(out=xjrep_sb[:], in_=bcast_src)

        ones3 = singles.tile([DIM, 1], f32)
        nc.vector.memset(ones3[:], 1.0)
        ones1_32 = singles.tile([1, H], f32)
        nc.vector.memset(ones1_32[:], 1.0)
        onesrow = singles.tile([1, NPACK], f32)
        nc.vector.memset(onesrow[:], 1.0)
        negb2row = singles.tile([1, P], f32)
        nc.vector.memset(negb2row[:], neg_eta_b2)

        def prep(b):
            # pair-grid prep on DVE (packed strips, free dim = 1280)
            dxs = dxp.tile([P, DIM, NPACK], f32, tag="dxs")
            for I in range(NCHUNK):
                for c in range(DIM):
                    nc.vector.tensor_scalar(
                        out=dxs[:, c, OFFS[I] : OFFS[I] + WIDTHS[I]],
                        in0=xjrep_sb[:, b, c, P * I : N],
                        scalar1=xi_sb[:, b, I, c : c + 1],
                        scalar2=None,
                        op0=OP.subtract,
                    )
            sqs = sqp.tile([P, DIM, NPACK], f32, tag="sqq")
            nc.vector.tensor_mul(sqs[:], dxs[:], dxs[:])
            d2s = d2p.tile([P, NPACK], f32, tag="d2s")
            nc.vector.reduce_sum(
                out=d2s[:], in_=sqs[:].rearrange("p c q -> p q c"), axis=AX.X
            )
            return d2s

        d2s_next = prep(0)
        for b in range(BPC):
            d2s = d2s_next
            # ---- sqrt phase (ACT, sqrt table set) ----
            ds = dqp.tile([P, NPACK], f32)
            nc.scalar.activation(ds[:], d2s[:], AF.Sqrt)
            # e_n: di = ||x_i|| as a [1, N] row
            sq3 = enp.tile([DIM, N], f32)
            nc.vector.tensor_mul(sq3[:], xT_sb[:, b, :], xT_sb[:, b, :])
            ni_ps = psen.tile([1, N], f32, tag="en")
            nc.tensor.matmul(ni_ps[:], ones3[:], sq3[:], start=True, stop=True)
            di = enp.tile([1, N], f32)
            nc.scalar.activation(di[:], ni_ps[:], AF.Sqrt)

            # ---- tanh phase (ACT tanh table set); M = -eta in PSUM ----
            acc = psacc.tile([P, NPACK], f32)
            for k in range(H):
                hs = hp.tile([P, NPACK], f32r)
                nc.scalar.activation(
                    hs[:],
                    ds[:],
                    AF.Tanh,
                    scale=etas_sb[:, 0, k : k + 1],
                    bias=etas_sb[:, 1, k : k + 1],
                )
                for off, w in MM_SPLITS:
                    nc.tensor.matmul(
                        acc[:, off : off + w],
                        w2diag_sb[:, k, :],
                        hs[:, off : off + w],
                        start=(k == 0),
                        stop=False,
                    )
            # -b2 into every entry: stationary -b2 row, moving all-ones row
            for off, w in MM_SPLITS:
                nc.tensor.matmul(
                    acc[:, off : off + w],
                    negb2row[:],
                    onesrow[:, off : off + w],
                    start=False,
                    stop=True,
                )
            # mu hidden layer on 32 partitions: tanh(w1*di + b1)
            direp_ps = psen.tile([H, N], f32, tag="en")
            nc.tensor.matmul(direp_ps[:], ones1_32[:], di[:], start=True, stop=True)
            hmu = enp.tile([H, N], f32)
            nc.scalar.activation(
                hmu[:],
                direp_ps[:],
                AF.Tanh,
                scale=mus_sb[:, 0:1],
                bias=mus_sb[:, 1:2],
            )
            mu_ps = psen.tile([DIM, N], f32, tag="en")
            nc.tensor.matmul(mu_ps[:], muw2_sb[:], hmu[:], start=True, stop=True)
            en = enp.tile([DIM, N], f32)
            nc.vector.scalar_tensor_tensor(
                out=en[:],
                in0=mu_ps[:],
                scalar=mu_b2_val,
                in1=xT_sb[:, b, :],
                op0=OP.add,
                op1=OP.mult,
            )

            # next batch's DVE prep before this batch's reflection, so the
            # scalar engine's next sqrt input is ready the moment tanh ends
            if b + 1 < BPC:
                d2s_next = prep(b + 1)

            # ---- reflection: P_c and Q rows via PE contractions ----
            acc_sb = accsbp.tile([P, NPACK], f32)
            nc.vector.tensor_copy(acc_sb[:], acc[:])

            def blk(I, J):
                off = OFFS[I] + (J - I) * P
                return acc_sb[:, off : off + P]

            # psum_out rows: x_c-weighted sums P_c and plain sums Q, separate
            # tiles because engine APs must start at partition 0
            poutP = psout.tile([DIM, NCHUNK, P], f32, tag="poutP")
            poutQ = psout.tile([DIM, NCHUNK, P], f32, tag="poutQ")
            # start=True resets PSUM state at bank granularity, so exactly one
            # start (the first matmul into each tile) and one stop (the last);
            # per-element has_written bits make later first-touches overwrite
            # and repeat-touches accumulate.
            ncontrib = [0]
            NTOT = NCHUNK * NCHUNK  # 16 contributions per tile

            def contrib(row_chunk, stat_chunk, mov_ap):
                g = ncontrib[0]
                ncontrib[0] += 1
                nc.tensor.matmul(
                    poutP[:, row_chunk, :],
                    statx[:, b, stat_chunk, 0:DIM],
                    mov_ap,
                    start=(g == 0),
                    stop=(g == NTOT - 1),
                    skip_group_check=True,
                )
                nc.tensor.matmul(
                    poutQ[:, row_chunk, :],
                    statx[:, b, stat_chunk, DIM : 2 * DIM],
                    mov_ap,
                    start=(g == 0),
                    stop=(g == NTOT - 1),
                    skip_group_check=True,
                )

            # diagonal blocks first (start=True for each row-chunk)
            for I in range(NCHUNK):
                contrib(I, I, blk(I, I))
            # off-diagonal: direct gives rows J; transposed gives rows I
            for I in range(NCHUNK):
                for J in range(I + 1, NCHUNK):
                    contrib(J, I, blk(I, J))
            for I in range(NCHUNK):
                for J in range(I + 1, NCHUNK):
                    tps = pstr.tile([P, P], f32)
                    nc.tensor.transpose(tps[:], blk(I, J), ident_sb[:])
                    at_sb = atp.tile([P, P], f32)
                    nc.vector.tensor_copy(at_sb[:], tps[:])
                    if debug_out and b == 0 and I == 0 and J == 1:
                        nc.gpsimd.dma_start(out=dbg_at_d[:], in_=at_sb[:])
                    contrib(I, J, at_sb[:])

            # ---- finalize: e_c = P_c - x_c*Q + e_n, in [c, i] layout ----
            outrow = orp.tile([DIM, N], f32)
            for I in range(NCHUNK):
                xq = enp.tile([DIM, P], f32, tag="xq")
                nc.vector.tensor_mul(
                    xq[:], xT_sb[:, b, I * P : (I + 1) * P], poutQ[:, I, :]
                )
                pm = enp.tile([DIM, P], f32, tag="pm")
                nc.vector.tensor_sub(pm[:], poutP[:, I, :], xq[:])
                nc.vector.tensor_add(
                    outrow[:, I * P : (I + 1) * P],
                    pm[:],
                    en[:, I * P : (I + 1) * P],
                )
            nc.gpsimd.dma_start(
                out=out_d[b].rearrange("i c -> c i"), in_=outrow[:]
            )
            if debug_out and b == 0:
                nc.gpsimd.dma_start(out=dbg_acc_d[:], in_=acc_sb[:])
                ppsb = orp.tile([DIM, NCHUNK, P], f32, tag="dbgpp")
                nc.vector.tensor_copy(ppsb[:], poutP[:])
                nc.gpsimd.dma_start(out=dbg_pp_d[:], in_=ppsb[:])
                pqsb = orp.tile([DIM, NCHUNK, P], f32, tag="dbgpq")
                nc.vector.tensor_copy(pqsb[:], poutQ[:])
                nc.gpsimd.dma_start(out=dbg_pq_d[:], in_=pqsb[:])

    _spread_sync_waits(nc)
    return nc


def _ensure_ntff_hook():
    """bass_utils' axon trace path imports antenv.axon_hooks, which the image's
    antenv package lacks. Register an equivalent module backed by the boot
    package's ctypes NTFF hook so trace=True works; degrade silently if the
    pieces are missing (tracing is optional)."""
    import os
    import types

    try:
        import antenv.axon_hooks  # noqa: F401

        return
    except ImportError:
        pass
    try:
        import antenv
    except ImportError:
        return
    mod = types.ModuleType("antenv.axon_hooks")
    box = {"h": None}
    mod.set_axon_ntff_profile_hook = lambda h: box.__setitem__("h", h)
    mod.get_axon_ntff_profile_hook = lambda: box["h"]
    sys.modules["antenv.axon_hooks"] = mod
    antenv.axon_hooks = mod
    try:
        from trn_agent_boot.trn_boot import _ntff_profile_via_ctypes

        so = "/opt/axon/libaxon_pjrt.so"
        if os.path.exists(so):
            hook = _ntff_profile_via_ctypes(so)
            if hook is not None:
                mod.set_axon_ntff_profile_hook(hook)
    except Exception:
        pass


def kernel(x, eta_w1, eta_b1, eta_w2, eta_b2, mu_w1, mu_b1, mu_w2, mu_b2):
    global LAST_RESULT
    _ensure_ntff_hook()
    from concourse.bass_utils import run_bass_kernel_spmd

    f32 = np.float32
    x = np.ascontiguousarray(np.asarray(x, dtype=f32))
    eta_w1 = np.asarray(eta_w1, f32)
    eta_b1 = np.asarray(eta_b1, f32)
    eta_w2 = np.asarray(eta_w2, f32)
    eta_b2 = np.asarray(eta_b2, f32)
    mu_w1 = np.asarray(mu_w1, f32)
    mu_b1 = np.asarray(mu_b1, f32)
    mu_w2 = np.asarray(mu_w2, f32)
    mu_b2 = np.asarray(mu_b2, f32)

    nc = _build_program(float(-eta_b2[0]), float(mu_b2[0]))

    w2diag = np.zeros((H, P, P), f32)
    idx = np.arange(P)
    w2diag[:, idx, idx] = -eta_w2[:, 0][:, None]
    etas = np.zeros((P, 2, H), f32)
    etas[:, 0, :] = eta_w1[0][None, :]
    etas[:, 1, :] = eta_b1[None, :]
    mus = np.stack([mu_w1[0], mu_b1], axis=1).astype(f32)  # [H, 2]
    muw2 = np.repeat(mu_w2, DIM, axis=1).astype(f32)  # [H, DIM]
    ident = np.eye(P, dtype=f32)

    in_maps = []
    for core in range(NCORES):
        xc = np.ascontiguousarray(x[core * BPC : (core + 1) * BPC])
        xTc = np.ascontiguousarray(xc.transpose(0, 2, 1))
        in_maps.append(
            {
                "x": xc,
                "xT": xTc,
                "w2diag": w2diag,
                "etas": etas,
                "mus": mus,
                "muw2": muw2,
                "ident": ident,
            }
        )

    res = run_bass_kernel_spmd(nc, in_maps, core_ids=list(range(NCORES)))
    LAST_RESULT = res
    out = np.concatenate([r["out"] for r in res.results], axis=0)
    return out.astype(np.float32)


# revision 19
# speedup vs baseline: 1.7825x; 1.0353x over previous
"""Trainium2 Bass kernel for the Backflow module.

Math (B=16, N=512, DIM=3, H=32):
  out[b,i,:] = sum_j eta(||x_bi - x_bj||) * (x_bi - x_bj)  +  mu(||x_bi||) * x_bi
where eta/mu are 1->H->1 tanh MLPs. The reference's eye()/diagonal correction
cancels exactly: the matrix form below includes the diagonal in both sums, and
eta(0)*(x_i - x_i) = 0.

Sharding: data-parallel over batch, 2 batches per core on 8 cores; the tiny
MLP parameters are replicated.

Per-core layout: i on partitions (4 chunks of 128), j on the free dim.
Symmetry eta(d_ij) = eta(d_ji): compute only block-triangular strips
(chunk I covers j in [128*I, 512)), packed to [128, 1280] (-37% tanh work).

  M[i,j] := -eta(d_ij) is built in PSUM: 32 tanh ACT ops over the packed strip
  (scale/bias = eta w1/b1 per k), each scaled by -w2_k via a PE matmul with
  stationary diag(-w2_k), plus a ones-matmul adding -b2. float32r is used on
  the matmul path (4x faster than fp32 at moving >= 256; ~tf32 precision).

  Row sums come from PE contractions with stationary [x_I | 1]:
    P_c[m] = sum_n M[m,n] x_c[n],  Q[m] = sum_n M[m,n]
    e_e_c[m] = sum_n eta*(x_c[m]-x_c[n]) = P_c[m] - x_c[m]*Q[m]
  Direct blocks give the (J,*) rows, PE-transposed blocks give the reflected
  (I,*) rows.

  ACT table sets: sqrt and tanh never share a set, so all Sqrt work of a batch
  is grouped before all Tanh work (2 table loads per batch).
"""

import sys

sys.path.insert(0, "/opt/trn_rl_repo")

import numpy as np
from contextlib import ExitStack

B, N, DIM, H = 16, 512, 3, 32
NCORES = 8
BPC = B // NCORES  # batches per core
P = 128
NCHUNK = N // P  # 4
# block-triangular strips: chunk I covers j in [128*I, N)
WIDTHS = [N - P * I for I in range(NCHUNK)]  # [512, 384, 256, 128]
OFFS = [0]
for w in WIDTHS[:-1]:
    OFFS.append(OFFS[-1] + w)
NPACK = sum(WIDTHS)  # 1280
# matmul column splits over the packed strip (N<=512, each >=256 for f32r)
MM_SPLITS = [(0, 512), (512, 512), (1024, 256)]

LAST_RESULT = None


def _spread_sync_waits(nc):
    """The pinned walrus rejects instructions carrying more than one sync wait
    ('Too many sync wait commands'). Engines execute their instruction streams
    in order, so hoist all-but-one wait of any such instruction onto same-engine
    NoOps inserted directly before it — semantically identical ordering."""
    from concourse import mybir

    n_added = 0
    for bb in nc.main_func.blocks:
        insts = bb.instructions
        i = 0
        while i < len(insts):
            inst = insts[i]
            si = getattr(inst, "sync_info", None)
            waits = list(si.on_wait) if si is not None and si.on_wait else []
            if len(waits) > 1:
                si.on_wait = waits[-1:]
                for k, w in enumerate(waits[:-1]):
                    nop = mybir.InstNoOp(
                        name=f"{inst.name}-wspread{k}",
                        sync_info=mybir.SyncInfo(on_wait=[w], on_update=[]),
                        engine=inst.engine,
                        bass_nofuse=True,
                    )
                    insts.insert(i + k, nop)
                    n_added += 1
                i += len(waits) - 1
            i += 1
    return n_added


def _build_program(neg_eta_b2: float, mu_b2_val: float, debug_out: bool = False):
    import concourse.bass as bass
    import concourse.tile as tile
    from concourse import mybir

    f32 = mybir.dt.float32
    f32r = mybir.dt.float32r
    AF = mybir.ActivationFunctionType
    OP = mybir.AluOpType
    AX = mybir.AxisListType

    nc = bass.Bass()
    x_d = nc.dram_tensor("x", [BPC, N, DIM], f32, kind="ExternalInput")
    xTn_d = nc.dram_tensor("xTn", [DIM + 1, BPC, N], f32, kind="ExternalInput")
    statd_d = nc.dram_tensor("statd", [DIM + 1, BPC, NCHUNK, P], f32, kind="ExternalInput")
    xin2_d = nc.dram_tensor("xin2", [P, BPC, NCHUNK], f32, kind="ExternalInput")
    w2diag_d = nc.dram_tensor("w2diag", [H, P, P], f32, kind="ExternalInput")
    etas_d = nc.dram_tensor("etas", [P, 2, H], f32, kind="ExternalInput")
    mus_d = nc.dram_tensor("mus", [H, 2], f32, kind="ExternalInput")
    muw2_d = nc.dram_tensor("muw2", [H, DIM], f32, kind="ExternalInput")
    ident_d = nc.dram_tensor("ident", [P, P], f32, kind="ExternalInput")
    out_d = nc.dram_tensor("out", [BPC, N, DIM], f32, kind="ExternalOutput")
    warm_d = nc.dram_tensor("warm", [1, 4], f32, kind="ExternalOutput")
    if debug_out:
        dbg_acc_d = nc.dram_tensor("dbg_acc", [P, NPACK], f32, kind="ExternalOutput")
        dbg_pp_d = nc.dram_tensor("dbg_pp", [DIM, NCHUNK, P], f32, kind="ExternalOutput")
        dbg_pq_d = nc.dram_tensor("dbg_pq", [DIM, NCHUNK, P], f32, kind="ExternalOutput")
        dbg_at_d = nc.dram_tensor("dbg_at", [P, P], f32, kind="ExternalOutput")

    with tile.TileContext(nc) as tc, ExitStack() as ctx:
        singles = ctx.enter_context(tc.tile_pool(name="singles", bufs=1))
        stgp = ctx.enter_context(tc.tile_pool(name="stgp", bufs=1))
        d2p = ctx.enter_context(tc.tile_pool(name="d2p", bufs=2))
        dqp = ctx.enter_context(tc.tile_pool(name="dqp", bufs=2))
        hp = ctx.enter_context(tc.tile_pool(name="hp", bufs=6))
        accsbp = ctx.enter_context(tc.tile_pool(name="accsbp", bufs=2))
        atp = ctx.enter_context(tc.tile_pool(name="atp", bufs=3))
        enp = ctx.enter_context(tc.tile_pool(name="enp", bufs=2))
        orp = ctx.enter_context(tc.tile_pool(name="orp", bufs=2))
        psacc = ctx.enter_context(tc.tile_pool(name="psacc", bufs=1, space="PSUM"))
        psout = ctx.enter_context(tc.tile_pool(name="psout", bufs=1, space="PSUM"))
        pstr = ctx.enter_context(tc.tile_pool(name="pstr", bufs=1, space="PSUM"))
        psd2 = ctx.enter_context(tc.tile_pool(name="psd2", bufs=2, space="PSUM"))

        # ---- constants / replicated inputs ----
        w2diag_st = stgp.tile([P, H, P], f32)
        nc.gpsimd.dma_start(
            out=w2diag_st[:], in_=w2diag_d[:].rearrange("k p m -> p k m")
        )
        w2diag_sb = singles.tile([P, H, P], f32r)
        nc.vector.tensor_copy(w2diag_sb[:], w2diag_st[:])
        etas_sb = singles.tile([P, 2, H], f32)
        nc.gpsimd.dma_start(out=etas_sb[:], in_=etas_d[:])
        mus_sb = singles.tile([H, 2], f32)
        nc.gpsimd.dma_start(out=mus_sb[:], in_=mus_d[:])
        muw2_sb = singles.tile([H, DIM], f32)
        nc.gpsimd.dma_start(out=muw2_sb[:], in_=muw2_d[:])
        ident_sb = singles.tile([P, P], f32)
        nc.gpsimd.dma_start(out=ident_sb[:], in_=ident_d[:])

        # reflection stationaries: [x_I cols | ones cols] per (b, I)
        statx = singles.tile([P, BPC, NCHUNK, 2 * DIM], f32)
        nc.gpsimd.dma_start(
            out=statx[:, :, :, 0:DIM],
            in_=x_d[:].rearrange("b (i p) c -> p b i c", p=P),
        )
        nc.vector.memset(statx[:, :, :, DIM : 2 * DIM], 1.0)
        # xT rows + ||x_j||^2 row (moving operand of the d^2 matmul; also e_n)
        xTn_sb = singles.tile([DIM + 1, BPC, N], f32)
        nc.gpsimd.dma_start(out=xTn_sb[:], in_=xTn_d[:])
        # d^2 matmul stationaries [-2 x_I | 1] and per-partition ||x_i||^2
        statd_sb = singles.tile([DIM + 1, BPC, NCHUNK, P], f32)
        nc.gpsimd.dma_start(out=statd_sb[:], in_=statd_d[:])
        xin2_sb = singles.tile([P, BPC, NCHUNK], f32)
        nc.gpsimd.dma_start(out=xin2_sb[:], in_=xin2_d[:])
        xn_sb = singles.tile([1, BPC, N], f32)
        nc.gpsimd.dma_start(out=xn_sb[:], in_=xTn_d[DIM : DIM + 1, :, :])

        ones1_32 = singles.tile([1, H], f32)
        nc.vector.memset(ones1_32[:], 1.0)
        onesrow = singles.tile([1, NPACK], f32)
        nc.vector.memset(onesrow[:], 1.0)
        negb2row = singles.tile([1, P], f32)
        nc.vector.memset(negb2row[:], neg_eta_b2)

        # ---- PE warm-up burst: ~8us of back-to-back matmuls so HAM reaches
        # K=8/8 before the real pipeline; anchored by a DRAM output so it
        # cannot be dead-code-eliminated.
        warm_ps = psd2.tile([P, N], f32, tag="d2")
        for wk in range(20):
            nc.tensor.matmul(
                warm_ps[:],
                w2diag_sb[:, wk % H, :],
                w2diag_sb[:, 0:4, :],
                start=(wk == 0),
                stop=(wk == 19),
            )
        warm_sb = enp.tile([1, 4], f32, tag="warm")
        nc.vector.tensor_copy(warm_sb[:], warm_ps[0:1, 0:4])
        nc.gpsimd.dma_start(out=warm_d[:], in_=warm_sb[:])

        def prep(b):
            # d^2 strips on the PE: d2[i,j] = -2 x_i.x_j + ||x_j||^2 (matmul)
            # then + ||x_i||^2 and clamp-at-0 fused in one dual-op
            # tensor_scalar per strip (guards sqrt against tiny negatives).
            d2s = d2p.tile([P, NPACK], f32, tag="d2s")
            for I in range(NCHUNK):
                d2ps = psd2.tile([P, WIDTHS[I]], f32, tag="d2")
                nc.tensor.matmul(
                    d2ps[:],
                    statd_sb[:, b, I, :],
                    xTn_sb[:, b, P * I : N],
                    start=True,
                    stop=True,
                )
                nc.vector.tensor_scalar(
                    out=d2s[:, OFFS[I] : OFFS[I] + WIDTHS[I]],
                    in0=d2ps[:],
                    scalar1=xin2_sb[:, b, I : I + 1],
                    scalar2=0.0,
                    op0=OP.add,
                    op1=OP.max,
                )
            return d2s

        d2s_next = prep(0)
        for b in range(BPC):
            d2s = d2s_next
            # ---- sqrt phase (ACT, sqrt table set) ----
            ds = dqp.tile([P, NPACK], f32)
            nc.scalar.activation(ds[:], d2s[:], AF.Sqrt)
            # e_n: di = ||x_i|| straight from the host-provided norm row
            di = enp.tile([1, N], f32)
            nc.scalar.activation(di[:], xn_sb[:, b, :], AF.Sqrt)

            # ---- tanh phase (ACT tanh table set); M = -eta in PSUM ----
            acc = psacc.tile([P, NPACK], f32)
            for k in range(H):
                hs = hp.tile([P, NPACK], f32r)
                nc.scalar.activation(
                    hs[:],
                    ds[:],
                    AF.Tanh,
                    scale=etas_sb[:, 0, k : k + 1],
                    bias=etas_sb[:, 1, k : k + 1],
                )
                for off, w in MM_SPLITS:
                    nc.tensor.matmul(
                        acc[:, off : off + w],
                        w2diag_sb[:, k, :],
                        hs[:, off : off + w],
                        start=(k == 0),
                        stop=False,
                    )
            # -b2 into every entry: stationary -b2 row, moving all-ones row
            for off, w in MM_SPLITS:
                nc.tensor.matmul(
                    acc[:, off : off + w],
                    negb2row[:],
                    onesrow[:, off : off + w],
                    start=False,
                    stop=True,
                )
            # mu hidden layer on 32 partitions: tanh(w1*di + b1)
            direp_ps = pstr.tile([H, N], f32, tag="en")
            nc.tensor.matmul(direp_ps[:], ones1_32[:], di[:], start=True, stop=True)
            hmu = enp.tile([H, N], f32)
            nc.scalar.activation(
                hmu[:],
                direp_ps[:],
                AF.Tanh,
                scale=mus_sb[:, 0:1],
                bias=mus_sb[:, 1:2],
            )
            mu_ps = pstr.tile([DIM, N], f32, tag="en")
            nc.tensor.matmul(mu_ps[:], muw2_sb[:], hmu[:], start=True, stop=True)
            en = enp.tile([DIM, N], f32)
            nc.vector.scalar_tensor_tensor(
                out=en[:],
                in0=mu_ps[:],
                scalar=mu_b2_val,
                in1=xTn_sb[0:DIM, b, :],
                op0=OP.add,
                op1=OP.mult,
            )

            # next batch's DVE prep before this batch's reflection, so the
            # scalar engine's next sqrt input is ready the moment tanh ends
            if b + 1 < BPC:
                d2s_next = prep(b + 1)

            # ---- reflection: P_c and Q rows via PE contractions ----
            acc_sb = accsbp.tile([P, NPACK], f32)
            nc.vector.tensor_copy(acc_sb[:], acc[:])

            def blk(I, J):
                off = OFFS[I] + (J - I) * P
                return acc_sb[:, off : off + P]

            # psum_out rows: x_c-weighted sums P_c and plain sums Q, separate
            # tiles because engine APs must start at partition 0
            poutP = psout.tile([DIM, NCHUNK, P], f32, tag="poutP")
            poutQ = psout.tile([DIM, NCHUNK, P], f32, tag="poutQ")
            # start=True resets PSUM state at bank granularity, so exactly one
            # start (the first matmul into each tile) and one stop (the last);
            # per-element has_written bits make later first-touches overwrite
            # and repeat-touches accumulate.
            ncontrib = [0]
            NTOT = NCHUNK * NCHUNK  # 16 contributions per tile

            def contrib(row_chunk, stat_chunk, mov_ap):
                g = ncontrib[0]
                ncontrib[0] += 1
                nc.tensor.matmul(
                    poutP[:, row_chunk, :],
                    statx[:, b, stat_chunk, 0:DIM],
                    mov_ap,
                    start=(g == 0),
                    stop=(g == NTOT - 1),
                    skip_group_check=True,
                )
                nc.tensor.matmul(
                    poutQ[:, row_chunk, :],
                    statx[:, b, stat_chunk, DIM : 2 * DIM],
                    mov_ap,
                    start=(g == 0),
                    stop=(g == NTOT - 1),
                    skip_group_check=True,
                )

            # diagonal blocks first (start=True for each row-chunk)
            for I in range(NCHUNK):
                contrib(I, I, blk(I, I))
            # off-diagonal: direct gives rows J; transposed gives rows I
            for I in range(NCHUNK):
                for J in range(I + 1, NCHUNK):
                    contrib(J, I, blk(I, J))
            for I in range(NCHUNK):
                for J in range(I + 1, NCHUNK):
                    tps = pstr.tile([P, P], f32, tag="en")
                    nc.tensor.transpose(tps[:], blk(I, J), ident_sb[:])
                    at_sb = atp.tile([P, P], f32)
                    nc.vector.tensor_copy(at_sb[:], tps[:])
                    if debug_out and b == 0 and I == 0 and J == 1:
                        nc.gpsimd.dma_start(out=dbg_at_d[:], in_=at_sb[:])
                    contrib(I, J, at_sb[:])

            # ---- finalize: e_c = P_c - x_c*Q + e_n, in [c, i] layout ----
            outrow = orp.tile([DIM, N], f32)
            for I in range(NCHUNK):
                xq = enp.tile([DIM, P], f32, tag="xq")
                nc.vector.tensor_mul(
                    xq[:], xTn_sb[0:DIM, b, I * P : (I + 1) * P], poutQ[:, I, :]
                )
                pm = enp.tile([DIM, P], f32, tag="pm")
                nc.vector.tensor_sub(pm[:], poutP[:, I, :], xq[:])
                nc.vector.tensor_add(
                    outrow[:, I * P : (I + 1) * P],
                    pm[:],
                    en[:, I * P : (I + 1) * P],
                )
            nc.gpsimd.dma_start(
                out=out_d[b].rearrange("i c -> c i"), in_=outrow[:]
            )
            if debug_out and b == 0:
                nc.gpsimd.dma_start(out=dbg_acc_d[:], in_=acc_sb[:])
                ppsb = orp.tile([DIM, NCHUNK, P], f32, tag="dbgpp")
                nc.vector.tensor_copy(ppsb[:], poutP[:])
                nc.gpsimd.dma_start(out=dbg_pp_d[:], in_=ppsb[:])
                pqsb = orp.tile([DIM, NCHUNK, P], f32, tag="dbgpq")
                nc.vector.tensor_copy(pqsb[:], poutQ[:])
                nc.gpsimd.dma_start(out=dbg_pq_d[:], in_=pqsb[:])

    _spread_sync_waits(nc)
    return nc


def _ensure_ntff_hook():
    """bass_utils' axon trace path imports antenv.axon_hooks, which the image's
    antenv package lacks. Register an equivalent module backed by the boot
    package's ctypes NTFF hook so trace=True works; degrade silently if the
    pieces are missing (tracing is optional)."""
    import os
    import types

    try:
        import antenv.axon_hooks  # noqa: F401

        return
    except ImportError:
        pass
    try:
        import antenv
    except ImportError:
        return
    mod = types.ModuleType("antenv.axon_hooks")
    box = {"h": None}
    mod.set_axon_ntff_profile_hook = lambda h: box.__setitem__("h", h)
    mod.get_axon_ntff_profile_hook = lambda: box["h"]
    sys.modules["antenv.axon_hooks"] = mod
    antenv.axon_hooks = mod
    try:
        from trn_agent_boot.trn_boot import _ntff_profile_via_ctypes

        so = "/opt/axon/libaxon_pjrt.so"
        if os.path.exists(so):
            hook = _ntff_profile_via_ctypes(so)
            if hook is not None:
                mod.set_axon_ntff_profile_hook(hook)
    except Exception:
        pass


def kernel(x, eta_w1, eta_b1, eta_w2, eta_b2, mu_w1, mu_b1, mu_w2, mu_b2):
    global LAST_RESULT
    _ensure_ntff_hook()
    from concourse.bass_utils import run_bass_kernel_spmd

    f32 = np.float32
    x = np.ascontiguousarray(np.asarray(x, dtype=f32))
    eta_w1 = np.asarray(eta_w1, f32)
    eta_b1 = np.asarray(eta_b1, f32)
    eta_w2 = np.asarray(eta_w2, f32)
    eta_b2 = np.asarray(eta_b2, f32)
    mu_w1 = np.asarray(mu_w1, f32)
    mu_b1 = np.asarray(mu_b1, f32)
    mu_w2 = np.asarray(mu_w2, f32)
    mu_b2 = np.asarray(mu_b2, f32)

    nc = _build_program(float(-eta_b2[0]), float(mu_b2[0]))

    w2diag = np.zeros((H, P, P), f32)
    idx = np.arange(P)
    w2diag[:, idx, idx] = -eta_w2[:, 0][:, None]
    etas = np.zeros((P, 2, H), f32)
    etas[:, 0, :] = eta_w1[0][None, :]
    etas[:, 1, :] = eta_b1[None, :]
    mus = np.stack([mu_w1[0], mu_b1], axis=1).astype(f32)  # [H, 2]
    muw2 = np.repeat(mu_w2, DIM, axis=1).astype(f32)  # [H, DIM]
    ident = np.eye(P, dtype=f32)

    in_maps = []
    for core in range(NCORES):
        xc = np.ascontiguousarray(x[core * BPC : (core + 1) * BPC])
        xTc = xc.transpose(0, 2, 1)  # [BPC, DIM, N]
        n2 = (xc ** 2).sum(axis=2)  # [BPC, N]
        xTn = np.concatenate(
            [xTc, n2[:, None, :]], axis=1
        ).transpose(1, 0, 2)  # [DIM+1, BPC, N]
        statd = np.empty((DIM + 1, BPC, NCHUNK, P), f32)
        xin2 = np.empty((P, BPC, NCHUNK), f32)
        for bb in range(BPC):
            for I in range(NCHUNK):
                statd[0:DIM, bb, I, :] = -2.0 * xTc[bb, :, I * P : (I + 1) * P]
                statd[DIM, bb, I, :] = 1.0
                xin2[:, bb, I] = n2[bb, I * P : (I + 1) * P]
        in_maps.append(
            {
                "x": xc,
                "xTn": np.ascontiguousarray(xTn),
                "statd": statd,
                "xin2": xin2,
                "w2diag": w2diag,
                "etas": etas,
                "mus": mus,
                "muw2": muw2,
                "ident": ident,
            }
        )

    res = run_bass_kernel_spmd(nc, in_maps, core_ids=list(range(NCORES)))
    LAST_RESULT = res
    out = np.concatenate([r["out"] for r in res.results], axis=0)
    return out.astype(np.float32)


# revision 21
# speedup vs baseline: 1.8928x; 1.0619x over previous
"""Trainium2 Bass kernel for the Backflow module.

Math (B=16, N=512, DIM=3, H=32):
  out[b,i,:] = sum_j eta(||x_bi - x_bj||) * (x_bi - x_bj)  +  mu(||x_bi||) * x_bi
where eta/mu are 1->H->1 tanh MLPs. The reference's eye()/diagonal correction
cancels exactly: the matrix form below includes the diagonal in both sums, and
eta(0)*(x_i - x_i) = 0.

Sharding: data-parallel over batch, 2 batches per core on 8 cores; the tiny
MLP parameters are replicated.

Per-core layout: i on partitions (4 chunks of 128), j on the free dim.
Symmetry eta(d_ij) = eta(d_ji): compute only block-triangular strips
(chunk I covers j in [128*I, 512)), packed to [128, 1280] (-37% tanh work).

  M[i,j] := -eta(d_ij) is built in PSUM: 32 tanh ACT ops over the packed strip
  (scale/bias = eta w1/b1 per k), each scaled by -w2_k via a PE matmul with
  stationary diag(-w2_k), plus a ones-matmul adding -b2. float32r is used on
  the matmul path (4x faster than fp32 at moving >= 256; ~tf32 precision).

  Row sums come from PE contractions with stationary [x_I | 1]:
    P_c[m] = sum_n M[m,n] x_c[n],  Q[m] = sum_n M[m,n]
    e_e_c[m] = sum_n eta*(x_c[m]-x_c[n]) = P_c[m] - x_c[m]*Q[m]
  Direct blocks give the (J,*) rows, PE-transposed blocks give the reflected
  (I,*) rows.

  ACT table sets: sqrt and tanh never share a set, so all Sqrt work of a batch
  is grouped before all Tanh work (2 table loads per batch).
"""

import sys

sys.path.insert(0, "/opt/trn_rl_repo")

import numpy as np
from contextlib import ExitStack

B, N, DIM, H = 16, 512, 3, 32
NCORES = 8
BPC = B // NCORES  # batches per core
P = 128
NCHUNK = N // P  # 4
# block-triangular strips: chunk I covers j in [128*I, N)
WIDTHS = [N - P * I for I in range(NCHUNK)]  # [512, 384, 256, 128]
OFFS = [0]
for w in WIDTHS[:-1]:
    OFFS.append(OFFS[-1] + w)
NPACK = sum(WIDTHS)  # 1280
# matmul column splits over the packed strip (N<=512, each >=256 for f32r)
MM_SPLITS = [(0, 512), (512, 512), (1024, 256)]

LAST_RESULT = None


def _spread_sync_waits(nc):
    """The pinned walrus rejects instructions carrying more than one sync wait
    ('Too many sync wait commands'). Engines execute their instruction streams
    in order, so hoist all-but-one wait of any such instruction onto same-engine
    NoOps inserted directly before it — semantically identical ordering."""
    from concourse import mybir

    n_added = 0
    for bb in nc.main_func.blocks:
        insts = bb.instructions
        i = 0
        while i < len(insts):
            inst = insts[i]
            si = getattr(inst, "sync_info", None)
            waits = list(si.on_wait) if si is not None and si.on_wait else []
            if len(waits) > 1:
                si.on_wait = waits[-1:]
                for k, w in enumerate(waits[:-1]):
                    nop = mybir.InstNoOp(
                        name=f"{inst.name}-wspread{k}",
                        sync_info=mybir.SyncInfo(on_wait=[w], on_update=[]),
                        engine=inst.engine,
                        bass_nofuse=True,
                    )
                    insts.insert(i + k, nop)
                    n_added += 1
                i += len(waits) - 1
            i += 1
    return n_added


def _build_program(neg_eta_b2: float, mu_b2_val: float, debug_out: bool = False):
    import concourse.bass as bass
    import concourse.tile as tile
    from concourse import mybir

    f32 = mybir.dt.float32
    f32r = mybir.dt.float32r
    AF = mybir.ActivationFunctionType
    OP = mybir.AluOpType
    AX = mybir.AxisListType

    nc = bass.Bass()
    x_d = nc.dram_tensor("x", [BPC, N, DIM], f32, kind="ExternalInput")
    xTn_d = nc.dram_tensor("xTn", [DIM + 1, BPC, N], f32, kind="ExternalInput")
    statd_d = nc.dram_tensor("statd", [DIM + 1, BPC, NCHUNK, P], f32, kind="ExternalInput")
    xin2_d = nc.dram_tensor("xin2", [P, BPC, NCHUNK], f32, kind="ExternalInput")
    w2diag_d = nc.dram_tensor("w2diag", [P, H, P], f32, kind="ExternalInput")
    etas_d = nc.dram_tensor("etas", [P, 2, H], f32, kind="ExternalInput")
    mus_d = nc.dram_tensor("mus", [H, 2], f32, kind="ExternalInput")
    muw2_d = nc.dram_tensor("muw2", [H, DIM], f32, kind="ExternalInput")
    ident_d = nc.dram_tensor("ident", [P, P], f32, kind="ExternalInput")
    out_d = nc.dram_tensor("out", [BPC, N, DIM], f32, kind="ExternalOutput")
    warm_d = nc.dram_tensor("warm", [1, 4], f32, kind="ExternalOutput")
    if debug_out:
        dbg_acc_d = nc.dram_tensor("dbg_acc", [P, NPACK], f32, kind="ExternalOutput")
        dbg_pp_d = nc.dram_tensor("dbg_pp", [DIM, NCHUNK, P], f32, kind="ExternalOutput")
        dbg_pq_d = nc.dram_tensor("dbg_pq", [DIM, NCHUNK, P], f32, kind="ExternalOutput")
        dbg_at_d = nc.dram_tensor("dbg_at", [P, P], f32, kind="ExternalOutput")

    with tile.TileContext(nc) as tc, ExitStack() as ctx:
        singles = ctx.enter_context(tc.tile_pool(name="singles", bufs=1))
        stgp = ctx.enter_context(tc.tile_pool(name="stgp", bufs=1))
        d2p = ctx.enter_context(tc.tile_pool(name="d2p", bufs=2))
        dqp = ctx.enter_context(tc.tile_pool(name="dqp", bufs=2))
        hp = ctx.enter_context(tc.tile_pool(name="hp", bufs=6))
        accsbp = ctx.enter_context(tc.tile_pool(name="accsbp", bufs=2))
        atp = ctx.enter_context(tc.tile_pool(name="atp", bufs=3))
        enp = ctx.enter_context(tc.tile_pool(name="enp", bufs=2))
        orp = ctx.enter_context(tc.tile_pool(name="orp", bufs=2))
        psacc = ctx.enter_context(tc.tile_pool(name="psacc", bufs=1, space="PSUM"))
        psout = ctx.enter_context(tc.tile_pool(name="psout", bufs=1, space="PSUM"))
        pstr = ctx.enter_context(tc.tile_pool(name="pstr", bufs=1, space="PSUM"))
        psd2 = ctx.enter_context(tc.tile_pool(name="psd2", bufs=2, space="PSUM"))

        # ---- constants / replicated inputs ----
        etas_sb = singles.tile([P, 2, H], f32)
        nc.gpsimd.dma_start(out=etas_sb[:], in_=etas_d[:])
        mus_sb = singles.tile([H, 2], f32)
        nc.gpsimd.dma_start(out=mus_sb[:], in_=mus_d[:])
        muw2_sb = singles.tile([H, DIM], f32)
        nc.gpsimd.dma_start(out=muw2_sb[:], in_=muw2_d[:])
        ident_sb = singles.tile([P, P], f32)
        nc.gpsimd.dma_start(out=ident_sb[:], in_=ident_d[:])

        # reflection stationaries: [x_I cols | ones cols] per (b, I)
        statx = singles.tile([P, BPC, NCHUNK, 2 * DIM], f32)
        nc.gpsimd.dma_start(
            out=statx[:, :, :, 0:DIM],
            in_=x_d[:].rearrange("b (i p) c -> p b i c", p=P),
        )
        nc.vector.memset(statx[:, :, :, DIM : 2 * DIM], 1.0)
        # xT rows + ||x_j||^2 row (moving operand of the d^2 matmul; also e_n)
        xTn_sb = singles.tile([DIM + 1, BPC, N], f32)
        nc.gpsimd.dma_start(out=xTn_sb[:], in_=xTn_d[:])
        # d^2 matmul stationaries [-2 x_I | 1] and per-partition ||x_i||^2
        statd_sb = singles.tile([DIM + 1, BPC, NCHUNK, P], f32)
        nc.gpsimd.dma_start(out=statd_sb[:], in_=statd_d[:])
        xin2_sb = singles.tile([P, BPC, NCHUNK], f32)
        nc.gpsimd.dma_start(out=xin2_sb[:], in_=xin2_d[:])
        xn_sb = singles.tile([1, BPC, N], f32)
        nc.gpsimd.dma_start(out=xn_sb[:], in_=xTn_d[DIM : DIM + 1, :, :])

        ones1_32 = singles.tile([1, H], f32)
        nc.vector.memset(ones1_32[:], 1.0)
        onesrow = singles.tile([1, NPACK], f32)
        nc.vector.memset(onesrow[:], 1.0)
        negb2row = singles.tile([1, P], f32)
        nc.vector.memset(negb2row[:], neg_eta_b2)

        # w2diag after the small latency-critical DMAs (contiguous layout)
        w2diag_sb = singles.tile([P, H, P], f32)
        nc.gpsimd.dma_start(out=w2diag_sb[:], in_=w2diag_d[:])
        w2diag_r = singles.tile([P, H, P], f32r)
        nc.vector.tensor_copy(w2diag_r[:], w2diag_sb[:])

        # ---- PE warm-up burst: ~8us of back-to-back matmuls so HAM reaches
        # K=8/8 before the real pipeline; DMA-independent (memset source) and
        # anchored by a DRAM output so it cannot be dead-code-eliminated.
        wsrc0 = singles.tile([P, N], f32)
        nc.vector.memset(wsrc0[:], 0.001)
        wsrc = singles.tile([P, N], f32r)
        nc.vector.tensor_copy(wsrc[:], wsrc0[:])
        warm_ps = psd2.tile([P, N], f32, tag="d2")
        for wk in range(20):
            nc.tensor.matmul(
                warm_ps[:],
                wsrc[:, 0:P],
                wsrc[:],
                start=(wk == 0),
                stop=(wk == 19),
            )
        warm_sb = enp.tile([1, 4], f32, tag="warm")
        nc.vector.tensor_copy(warm_sb[:], warm_ps[0:1, 0:4])
        nc.gpsimd.dma_start(out=warm_d[:], in_=warm_sb[:])

        def prep(b):
            # d^2 strips on the PE: d2[i,j] = -2 x_i.x_j + ||x_j||^2 (matmul)
            # then + ||x_i||^2 and clamp-at-0 fused in one dual-op
            # tensor_scalar per strip (guards sqrt against tiny negatives).
            d2s = d2p.tile([P, NPACK], f32, tag="d2s")
            for I in range(NCHUNK):
                d2ps = psd2.tile([P, WIDTHS[I]], f32, tag="d2")
                nc.tensor.matmul(
                    d2ps[:],
                    statd_sb[:, b, I, :],
                    xTn_sb[:, b, P * I : N],
                    start=True,
                    stop=True,
                )
                nc.vector.tensor_scalar(
                    out=d2s[:, OFFS[I] : OFFS[I] + WIDTHS[I]],
                    in0=d2ps[:],
                    scalar1=xin2_sb[:, b, I : I + 1],
                    scalar2=0.0,
                    op0=OP.add,
                    op1=OP.max,
                )
            return d2s

        d2s_next = prep(0)
        for b in range(BPC):
            d2s = d2s_next
            # ---- sqrt phase (ACT, sqrt table set) ----
            ds = dqp.tile([P, NPACK], f32)
            nc.scalar.activation(ds[:], d2s[:], AF.Sqrt)
            # e_n: di = ||x_i|| straight from the host-provided norm row
            di = enp.tile([1, N], f32)
            nc.scalar.activation(di[:], xn_sb[:, b, :], AF.Sqrt)

            # ---- tanh phase (ACT tanh table set); M = -eta in PSUM ----
            acc = psacc.tile([P, NPACK], f32)
            for k in range(H):
                hs = hp.tile([P, NPACK], f32r)
                nc.scalar.activation(
                    hs[:],
                    ds[:],
                    AF.Tanh,
                    scale=etas_sb[:, 0, k : k + 1],
                    bias=etas_sb[:, 1, k : k + 1],
                )
                for off, w in MM_SPLITS:
                    nc.tensor.matmul(
                        acc[:, off : off + w],
                        w2diag_r[:, k, :],
                        hs[:, off : off + w],
                        start=(k == 0),
                        stop=False,
                    )
            # -b2 into every entry: stationary -b2 row, moving all-ones row
            for off, w in MM_SPLITS:
                nc.tensor.matmul(
                    acc[:, off : off + w],
                    negb2row[:],
                    onesrow[:, off : off + w],
                    start=False,
                    stop=True,
                )
            # mu hidden layer on 32 partitions: tanh(w1*di + b1)
            direp_ps = pstr.tile([H, N], f32, tag="en")
            nc.tensor.matmul(direp_ps[:], ones1_32[:], di[:], start=True, stop=True)
            hmu = enp.tile([H, N], f32)
            nc.scalar.activation(
                hmu[:],
                direp_ps[:],
                AF.Tanh,
                scale=mus_sb[:, 0:1],
                bias=mus_sb[:, 1:2],
            )
            mu_ps = pstr.tile([DIM, N], f32, tag="en")
            nc.tensor.matmul(mu_ps[:], muw2_sb[:], hmu[:], start=True, stop=True)
            en = enp.tile([DIM, N], f32)
            nc.vector.scalar_tensor_tensor(
                out=en[:],
                in0=mu_ps[:],
                scalar=mu_b2_val,
                in1=xTn_sb[0:DIM, b, :],
                op0=OP.add,
                op1=OP.mult,
            )

            # next batch's DVE prep before this batch's reflection, so the
            # scalar engine's next sqrt input is ready the moment tanh ends
            if b + 1 < BPC:
                d2s_next = prep(b + 1)

            # ---- reflection: P_c and Q rows via PE contractions ----
            acc_sb = accsbp.tile([P, NPACK], f32)
            for off, w in MM_SPLITS:
                nc.vector.tensor_copy(acc_sb[:, off : off + w], acc[:, off : off + w])

            def blk(I, J):
                off = OFFS[I] + (J - I) * P
                return acc_sb[:, off : off + P]

            # psum_out rows: x_c-weighted sums P_c and plain sums Q, separate
            # tiles because engine APs must start at partition 0
            poutP = psout.tile([DIM, NCHUNK, P], f32, tag="poutP")
            poutQ = psout.tile([DIM, NCHUNK, P], f32, tag="poutQ")
            # start=True resets PSUM state at bank granularity, so exactly one
            # start (the first matmul into each tile) and one stop (the last);
            # per-element has_written bits make later first-touches overwrite
            # and repeat-touches accumulate.
            ncontrib = [0]
            NTOT = NCHUNK * NCHUNK  # 16 contributions per tile

            def contrib(row_chunk, stat_chunk, mov_ap):
                g = ncontrib[0]
                ncontrib[0] += 1
                nc.tensor.matmul(
                    poutP[:, row_chunk, :],
                    statx[:, b, stat_chunk, 0:DIM],
                    mov_ap,
                    start=(g == 0),
                    stop=(g == NTOT - 1),
                    skip_group_check=True,
                )
                nc.tensor.matmul(
                    poutQ[:, row_chunk, :],
                    statx[:, b, stat_chunk, DIM : 2 * DIM],
                    mov_ap,
                    start=(g == 0),
                    stop=(g == NTOT - 1),
                    skip_group_check=True,
                )

            # diagonal blocks first (start=True for each row-chunk)
            for I in range(NCHUNK):
                contrib(I, I, blk(I, I))
            # off-diagonal: direct gives rows J; transposed gives rows I
            for I in range(NCHUNK):
                for J in range(I + 1, NCHUNK):
                    contrib(J, I, blk(I, J))
            for I in range(NCHUNK):
                for J in range(I + 1, NCHUNK):
                    tps = psd2.tile([P, P], f32, tag="d2")
                    nc.tensor.transpose(tps[:], blk(I, J), ident_sb[:])
                    at_sb = atp.tile([P, P], f32)
                    nc.vector.tensor_copy(at_sb[:], tps[:])
                    if debug_out and b == 0 and I == 0 and J == 1:
                        nc.gpsimd.dma_start(out=dbg_at_d[:], in_=at_sb[:])
                    contrib(I, J, at_sb[:])

            # ---- finalize: e_c = P_c - x_c*Q + e_n, in [c, i] layout ----
            outrow = orp.tile([DIM, N], f32)
            for I in range(NCHUNK):
                xq = enp.tile([DIM, P], f32, tag="xq")
                nc.vector.tensor_mul(
                    xq[:], xTn_sb[0:DIM, b, I * P : (I + 1) * P], poutQ[:, I, :]
                )
                pm = enp.tile([DIM, P], f32, tag="pm")
                nc.vector.tensor_sub(pm[:], poutP[:, I, :], xq[:])
                nc.vector.tensor_add(
                    outrow[:, I * P : (I + 1) * P],
                    pm[:],
                    en[:, I * P : (I + 1) * P],
                )
            nc.gpsimd.dma_start(
                out=out_d[b].rearrange("i c -> c i"), in_=outrow[:]
            )
            if debug_out and b == 0:
                nc.gpsimd.dma_start(out=dbg_acc_d[:], in_=acc_sb[:])
                ppsb = orp.tile([DIM, NCHUNK, P], f32, tag="dbgpp")
                nc.vector.tensor_copy(ppsb[:], poutP[:])
                nc.gpsimd.dma_start(out=dbg_pp_d[:], in_=ppsb[:])
                pqsb = orp.tile([DIM, NCHUNK, P], f32, tag="dbgpq")
                nc.vector.tensor_copy(pqsb[:], poutQ[:])
                nc.gpsimd.dma_start(out=dbg_pq_d[:], in_=pqsb[:])

    _spread_sync_waits(nc)
    return nc


def _ensure_ntff_hook():
    """bass_utils' axon trace path imports antenv.axon_hooks, which the image's
    antenv package lacks. Register an equivalent module backed by the boot
    package's ctypes NTFF hook so trace=True works; degrade silently if the
    pieces are missing (tracing is optional)."""
    import os
    import types

    try:
        import antenv.axon_hooks  # noqa: F401

        return
    except ImportError:
        pass
    try:
        import antenv
    except ImportError:
        return
    mod = types.ModuleType("antenv.axon_hooks")
    box = {"h": None}
    mod.set_axon_ntff_profile_hook = lambda h: box.__setitem__("h", h)
    mod.get_axon_ntff_profile_hook = lambda: box["h"]
    sys.modules["antenv.axon_hooks"] = mod
    antenv.axon_hooks = mod
    try:
        from trn_agent_boot.trn_boot import _ntff_profile_via_ctypes

        so = "/opt/axon/libaxon_pjrt.so"
        if os.path.exists(so):
            hook = _ntff_profile_via_ctypes(so)
            if hook is not None:
                mod.set_axon_ntff_profile_hook(hook)
    except Exception:
        pass


def kernel(x, eta_w1, eta_b1, eta_w2, eta_b2, mu_w1, mu_b1, mu_w2, mu_b2):
    global LAST_RESULT
    _ensure_ntff_hook()
    from concourse.bass_utils import run_bass_kernel_spmd

    f32 = np.float32
    x = np.ascontiguousarray(np.asarray(x, dtype=f32))
    eta_w1 = np.asarray(eta_w1, f32)
    eta_b1 = np.asarray(eta_b1, f32)
    eta_w2 = np.asarray(eta_w2, f32)
    eta_b2 = np.asarray(eta_b2, f32)
    mu_w1 = np.asarray(mu_w1, f32)
    mu_b1 = np.asarray(mu_b1, f32)
    mu_w2 = np.asarray(mu_w2, f32)
    mu_b2 = np.asarray(mu_b2, f32)

    nc = _build_program(float(-eta_b2[0]), float(mu_b2[0]))

    w2diag = np.zeros((P, H, P), f32)
    idx = np.arange(P)
    w2diag[idx, :, idx] = -eta_w2[:, 0][None, :]
    etas = np.zeros((P, 2, H), f32)
    etas[:, 0, :] = eta_w1[0][None, :]
    etas[:, 1, :] = eta_b1[None, :]
    mus = np.stack([mu_w1[0], mu_b1], axis=1).astype(f32)  # [H, 2]
    muw2 = np.repeat(mu_w2, DIM, axis=1).astype(f32)  # [H, DIM]
    ident = np.eye(P, dtype=f32)

    in_maps = []
    for core in range(NCORES):
        xc = np.ascontiguousarray(x[core * BPC : (core + 1) * BPC])
        xTc = xc.transpose(0, 2, 1)  # [BPC, DIM, N]
        n2 = (xc ** 2).sum(axis=2)  # [BPC, N]
        xTn = np.concatenate(
            [xTc, n2[:, None, :]], axis=1
        ).transpose(1, 0, 2)  # [DIM+1, BPC, N]
        statd = np.empty((DIM + 1, BPC, NCHUNK, P), f32)
        xin2 = np.empty((P, BPC, NCHUNK), f32)
        for bb in range(BPC):
            for I in range(NCHUNK):
                statd[0:DIM, bb, I, :] = -2.0 * xTc[bb, :, I * P : (I + 1) * P]
                statd[DIM, bb, I, :] = 1.0
                xin2[:, bb, I] = n2[bb, I * P : (I + 1) * P]
        in_maps.append(
            {
                "x": xc,
                "xTn": np.ascontiguousarray(xTn),
                "statd": statd,
                "xin2": xin2,
                "w2diag": w2diag,
                "etas": etas,
                "mus": mus,
                "muw2": muw2,
                "ident": ident,
            }
        )

    res = run_bass_kernel_spmd(nc, in_maps, core_ids=list(range(NCORES)))
    LAST_RESULT = res
    out = np.concatenate([r["out"] for r in res.results], axis=0)
    return out.astype(np.float32)


# revision 22
# speedup vs baseline: 2.2196x; 1.1726x over previous
"""Trainium2 Bass kernel for the Backflow module.

Math (B=16, N=512, DIM=3, H=32):
  out[b,i,:] = sum_j eta(||x_bi - x_bj||) * (x_bi - x_bj)  +  mu(||x_bi||) * x_bi
where eta/mu are 1->H->1 tanh MLPs. The reference's eye()/diagonal correction
cancels exactly: the matrix form below includes the diagonal in both sums, and
eta(0)*(x_i - x_i) = 0.

Sharding: data-parallel over batch, 2 batches per core on 8 cores; the tiny
MLP parameters are replicated.

Per-core layout: i on partitions (4 chunks of 128), j on the free dim.
Symmetry eta(d_ij) = eta(d_ji): compute only block-triangular strips
(chunk I covers j in [128*I, 512)), packed to [128, 1280] (-37% tanh work).

  M[i,j] := -eta(d_ij) is built in PSUM: 32 tanh ACT ops over the packed strip
  (scale/bias = eta w1/b1 per k), each scaled by -w2_k via a PE matmul with
  stationary diag(-w2_k), plus a ones-matmul adding -b2. float32r is used on
  the matmul path (4x faster than fp32 at moving >= 256; ~tf32 precision).

  Row sums come from PE contractions with stationary [x_I | 1]:
    P_c[m] = sum_n M[m,n] x_c[n],  Q[m] = sum_n M[m,n]
    e_e_c[m] = sum_n eta*(x_c[m]-x_c[n]) = P_c[m] - x_c[m]*Q[m]
  Direct blocks give the (J,*) rows, PE-transposed blocks give the reflected
  (I,*) rows.

  ACT table sets: sqrt and tanh never share a set, so all Sqrt work of a batch
  is grouped before all Tanh work (2 table loads per batch).
"""

import sys

sys.path.insert(0, "/opt/trn_rl_repo")

import numpy as np
from contextlib import ExitStack

B, N, DIM, H = 16, 512, 3, 32
NCORES = 8
BPC = B // NCORES  # batches per core
P = 128
NCHUNK = N // P  # 4
# block-triangular strips: chunk I covers j in [128*I, N)
WIDTHS = [N - P * I for I in range(NCHUNK)]  # [512, 384, 256, 128]
OFFS = [0]
for w in WIDTHS[:-1]:
    OFFS.append(OFFS[-1] + w)
NPACK = sum(WIDTHS)  # 1280
# matmul column splits over the packed strip (N<=512, each >=256 for f32r)
MM_SPLITS = [(0, 512), (512, 512), (1024, 256)]

LAST_RESULT = None


def _spread_sync_waits(nc):
    """The pinned walrus rejects instructions carrying more than one sync wait
    ('Too many sync wait commands'). Engines execute their instruction streams
    in order, so hoist all-but-one wait of any such instruction onto same-engine
    NoOps inserted directly before it — semantically identical ordering."""
    from concourse import mybir

    n_added = 0
    for bb in nc.main_func.blocks:
        insts = bb.instructions
        i = 0
        while i < len(insts):
            inst = insts[i]
            si = getattr(inst, "sync_info", None)
            waits = list(si.on_wait) if si is not None and si.on_wait else []
            if len(waits) > 1:
                si.on_wait = waits[-1:]
                for k, w in enumerate(waits[:-1]):
                    nop = mybir.InstNoOp(
                        name=f"{inst.name}-wspread{k}",
                        sync_info=mybir.SyncInfo(on_wait=[w], on_update=[]),
                        engine=inst.engine,
                        bass_nofuse=True,
                    )
                    insts.insert(i + k, nop)
                    n_added += 1
                i += len(waits) - 1
            i += 1
    return n_added


def _build_program(neg_eta_b2: float, mu_b2_val: float, debug_out: bool = False):
    import concourse.bass as bass
    import concourse.tile as tile
    from concourse import mybir

    f32 = mybir.dt.float32
    f32r = mybir.dt.float32r
    AF = mybir.ActivationFunctionType
    OP = mybir.AluOpType
    AX = mybir.AxisListType

    nc = bass.Bass()
    x_d = nc.dram_tensor("x", [BPC, N, DIM], f32, kind="ExternalInput")
    xTn_d = nc.dram_tensor("xTn", [DIM + 1, BPC, N], f32, kind="ExternalInput")
    statd_d = nc.dram_tensor("statd", [DIM + 1, BPC, NCHUNK, P], f32, kind="ExternalInput")
    xin2_d = nc.dram_tensor("xin2", [P, BPC, NCHUNK], f32, kind="ExternalInput")
    w2diag_d = nc.dram_tensor("w2diag", [P, H, P], f32, kind="ExternalInput")
    etas_d = nc.dram_tensor("etas", [P, 2, H], f32, kind="ExternalInput")
    mus_d = nc.dram_tensor("mus", [H, 2], f32, kind="ExternalInput")
    muw2_d = nc.dram_tensor("muw2", [H, DIM], f32, kind="ExternalInput")
    ident_d = nc.dram_tensor("ident", [P, P], f32, kind="ExternalInput")
    out_d = nc.dram_tensor("out", [BPC, DIM, N], f32, kind="ExternalOutput")
    warm_d = nc.dram_tensor("warm", [1, 4], f32, kind="ExternalOutput")
    if debug_out:
        dbg_acc_d = nc.dram_tensor("dbg_acc", [P, NPACK], f32, kind="ExternalOutput")
        dbg_pp_d = nc.dram_tensor("dbg_pp", [DIM, NCHUNK, P], f32, kind="ExternalOutput")
        dbg_pq_d = nc.dram_tensor("dbg_pq", [DIM, NCHUNK, P], f32, kind="ExternalOutput")
        dbg_at_d = nc.dram_tensor("dbg_at", [P, P], f32, kind="ExternalOutput")

    with tile.TileContext(nc) as tc, ExitStack() as ctx:
        singles = ctx.enter_context(tc.tile_pool(name="singles", bufs=1))
        stgp = ctx.enter_context(tc.tile_pool(name="stgp", bufs=1))
        d2p = ctx.enter_context(tc.tile_pool(name="d2p", bufs=2))
        dqp = ctx.enter_context(tc.tile_pool(name="dqp", bufs=2))
        hp = ctx.enter_context(tc.tile_pool(name="hp", bufs=6))
        accsbp = ctx.enter_context(tc.tile_pool(name="accsbp", bufs=2))
        atp = ctx.enter_context(tc.tile_pool(name="atp", bufs=3))
        enp = ctx.enter_context(tc.tile_pool(name="enp", bufs=2))
        orp = ctx.enter_context(tc.tile_pool(name="orp", bufs=2))
        psacc = ctx.enter_context(tc.tile_pool(name="psacc", bufs=1, space="PSUM"))
        psout = ctx.enter_context(tc.tile_pool(name="psout", bufs=1, space="PSUM"))
        pstr = ctx.enter_context(tc.tile_pool(name="pstr", bufs=1, space="PSUM"))
        psd2 = ctx.enter_context(tc.tile_pool(name="psd2", bufs=2, space="PSUM"))

        # ---- constants / replicated inputs ----
        etas_sb = singles.tile([P, 2, H], f32)
        nc.gpsimd.dma_start(out=etas_sb[:], in_=etas_d[:])
        mus_sb = singles.tile([H, 2], f32)
        nc.gpsimd.dma_start(out=mus_sb[:], in_=mus_d[:])
        muw2_sb = singles.tile([H, DIM], f32)
        nc.gpsimd.dma_start(out=muw2_sb[:], in_=muw2_d[:])
        ident_sb = singles.tile([P, P], f32)
        nc.gpsimd.dma_start(out=ident_sb[:], in_=ident_d[:])

        # reflection stationaries: [x_I cols | ones cols] per (b, I)
        statx = singles.tile([P, BPC, NCHUNK, 2 * DIM], f32)
        nc.gpsimd.dma_start(
            out=statx[:, :, :, 0:DIM],
            in_=x_d[:].rearrange("b (i p) c -> p b i c", p=P),
        )
        nc.vector.memset(statx[:, :, :, DIM : 2 * DIM], 1.0)
        # xT rows + ||x_j||^2 row (moving operand of the d^2 matmul; also e_n)
        xTn_sb = singles.tile([DIM + 1, BPC, N], f32)
        nc.gpsimd.dma_start(out=xTn_sb[:], in_=xTn_d[:])
        # d^2 matmul stationaries [-2 x_I | 1] and per-partition ||x_i||^2
        statd_sb = singles.tile([DIM + 1, BPC, NCHUNK, P], f32)
        nc.gpsimd.dma_start(out=statd_sb[:], in_=statd_d[:])
        xin2_sb = singles.tile([P, BPC, NCHUNK], f32)
        nc.gpsimd.dma_start(out=xin2_sb[:], in_=xin2_d[:])
        xn_sb = singles.tile([1, BPC, N], f32)
        nc.gpsimd.dma_start(out=xn_sb[:], in_=xTn_d[DIM : DIM + 1, :, :])

        ones1_32 = singles.tile([1, H], f32)
        nc.vector.memset(ones1_32[:], 1.0)
        onesrow = singles.tile([1, NPACK], f32)
        nc.vector.memset(onesrow[:], 1.0)
        negb2row = singles.tile([1, P], f32)
        nc.vector.memset(negb2row[:], neg_eta_b2)

        # w2diag after the small latency-critical DMAs (contiguous layout)
        w2diag_sb = singles.tile([P, H, P], f32)
        nc.gpsimd.dma_start(out=w2diag_sb[:], in_=w2diag_d[:])
        w2diag_r = singles.tile([P, H, P], f32r)
        nc.vector.tensor_copy(w2diag_r[:], w2diag_sb[:])

        # ---- PE warm-up burst: ~8us of back-to-back matmuls so HAM reaches
        # K=8/8 before the real pipeline; DMA-independent (memset source) and
        # anchored by a DRAM output so it cannot be dead-code-eliminated.
        wsrc0 = singles.tile([P, N], f32)
        nc.vector.memset(wsrc0[:], 0.001)
        wsrc = singles.tile([P, N], f32r)
        nc.vector.tensor_copy(wsrc[:], wsrc0[:])
        warm_ps = psd2.tile([P, N], f32, tag="d2")
        for wk in range(8):
            nc.tensor.matmul(
                warm_ps[:],
                wsrc[:, 0:P],
                wsrc[:],
                start=(wk == 0),
                stop=(wk == 7),
            )
        warm_sb = enp.tile([1, 4], f32, tag="warm")
        nc.vector.tensor_copy(warm_sb[:], warm_ps[0:1, 0:4])
        nc.gpsimd.dma_start(out=warm_d[:], in_=warm_sb[:])

        def prep(b):
            # d^2 strips on the PE: d2[i,j] = -2 x_i.x_j + ||x_j||^2 (matmul)
            # then + ||x_i||^2 and clamp-at-0 fused in one dual-op
            # tensor_scalar per strip (guards sqrt against tiny negatives).
            d2s = d2p.tile([P, NPACK], f32, tag="d2s")
            for I in range(NCHUNK):
                d2ps = psd2.tile([P, WIDTHS[I]], f32, tag="d2")
                nc.tensor.matmul(
                    d2ps[:],
                    statd_sb[:, b, I, :],
                    xTn_sb[:, b, P * I : N],
                    start=True,
                    stop=True,
                )
                nc.vector.tensor_scalar(
                    out=d2s[:, OFFS[I] : OFFS[I] + WIDTHS[I]],
                    in0=d2ps[:],
                    scalar1=xin2_sb[:, b, I : I + 1],
                    scalar2=0.0,
                    op0=OP.add,
                    op1=OP.max,
                )
            return d2s

        d2s_next = prep(0)
        for b in range(BPC):
            d2s = d2s_next
            # ---- sqrt phase (ACT, sqrt table set) ----
            ds = dqp.tile([P, NPACK], f32)
            nc.scalar.activation(ds[:], d2s[:], AF.Sqrt)
            # e_n: di = ||x_i|| straight from the host-provided norm row
            di = enp.tile([1, N], f32)
            nc.scalar.activation(di[:], xn_sb[:, b, :], AF.Sqrt)

            # ---- tanh phase (ACT tanh table set); M = -eta in PSUM ----
            acc = psacc.tile([P, NPACK], f32)
            for k in range(H):
                hs = hp.tile([P, NPACK], f32r)
                nc.scalar.activation(
                    hs[:],
                    ds[:],
                    AF.Tanh,
                    scale=etas_sb[:, 0, k : k + 1],
                    bias=etas_sb[:, 1, k : k + 1],
                )
                for off, w in MM_SPLITS:
                    nc.tensor.matmul(
                        acc[:, off : off + w],
                        w2diag_r[:, k, :],
                        hs[:, off : off + w],
                        start=(k == 0),
                        stop=False,
                    )
            # -b2 into every entry: stationary -b2 row, moving all-ones row
            for off, w in MM_SPLITS:
                nc.tensor.matmul(
                    acc[:, off : off + w],
                    negb2row[:],
                    onesrow[:, off : off + w],
                    start=False,
                    stop=True,
                )
            # mu hidden layer on 32 partitions: tanh(w1*di + b1)
            direp_ps = pstr.tile([H, N], f32, tag="en")
            nc.tensor.matmul(direp_ps[:], ones1_32[:], di[:], start=True, stop=True)
            hmu = enp.tile([H, N], f32)
            nc.scalar.activation(
                hmu[:],
                direp_ps[:],
                AF.Tanh,
                scale=mus_sb[:, 0:1],
                bias=mus_sb[:, 1:2],
            )
            mu_ps = pstr.tile([DIM, N], f32, tag="en")
            nc.tensor.matmul(mu_ps[:], muw2_sb[:], hmu[:], start=True, stop=True)
            en = enp.tile([DIM, N], f32)
            nc.vector.scalar_tensor_tensor(
                out=en[:],
                in0=mu_ps[:],
                scalar=mu_b2_val,
                in1=xTn_sb[0:DIM, b, :],
                op0=OP.add,
                op1=OP.mult,
            )

            # next batch's DVE prep before this batch's reflection, so the
            # scalar engine's next sqrt input is ready the moment tanh ends
            if b + 1 < BPC:
                d2s_next = prep(b + 1)

            # ---- reflection: P_c and Q rows via PE contractions ----
            acc_sb = accsbp.tile([P, NPACK], f32)
            for off, w in MM_SPLITS:
                nc.vector.tensor_copy(acc_sb[:, off : off + w], acc[:, off : off + w])

            def blk(I, J):
                off = OFFS[I] + (J - I) * P
                return acc_sb[:, off : off + P]

            # psum_out rows: x_c-weighted sums P_c and plain sums Q, separate
            # tiles because engine APs must start at partition 0
            poutP = psout.tile([DIM, NCHUNK, P], f32, tag="poutP")
            poutQ = psout.tile([DIM, NCHUNK, P], f32, tag="poutQ")
            # start=True resets PSUM state at bank granularity, so exactly one
            # start (the first matmul into each tile) and one stop (the last);
            # per-element has_written bits make later first-touches overwrite
            # and repeat-touches accumulate.
            ncontrib = [0]
            NTOT = NCHUNK * NCHUNK  # 16 contributions per tile

            def contrib(row_chunk, stat_chunk, mov_ap):
                g = ncontrib[0]
                ncontrib[0] += 1
                nc.tensor.matmul(
                    poutP[:, row_chunk, :],
                    statx[:, b, stat_chunk, 0:DIM],
                    mov_ap,
                    start=(g == 0),
                    stop=(g == NTOT - 1),
                    skip_group_check=True,
                )
                nc.tensor.matmul(
                    poutQ[:, row_chunk, :],
                    statx[:, b, stat_chunk, DIM : 2 * DIM],
                    mov_ap,
                    start=(g == 0),
                    stop=(g == NTOT - 1),
                    skip_group_check=True,
                )

            # diagonal blocks first (start=True for each row-chunk)
            for I in range(NCHUNK):
                contrib(I, I, blk(I, I))
            # off-diagonal: direct gives rows J; transposed gives rows I
            for I in range(NCHUNK):
                for J in range(I + 1, NCHUNK):
                    contrib(J, I, blk(I, J))
            for I in range(NCHUNK):
                for J in range(I + 1, NCHUNK):
                    tps = psd2.tile([P, P], f32, tag="d2")
                    nc.tensor.transpose(tps[:], blk(I, J), ident_sb[:])
                    at_sb = atp.tile([P, P], f32)
                    nc.vector.tensor_copy(at_sb[:], tps[:])
                    if debug_out and b == 0 and I == 0 and J == 1:
                        nc.gpsimd.dma_start(out=dbg_at_d[:], in_=at_sb[:])
                    contrib(I, J, at_sb[:])

            # ---- finalize: e_c = P_c - x_c*Q + e_n, in [c, i] layout ----
            outrow = orp.tile([DIM, N], f32)
            for I in range(NCHUNK):
                xq = enp.tile([DIM, P], f32, tag="xq")
                nc.vector.tensor_mul(
                    xq[:], xTn_sb[0:DIM, b, I * P : (I + 1) * P], poutQ[:, I, :]
                )
                pm = enp.tile([DIM, P], f32, tag="pm")
                nc.vector.tensor_sub(pm[:], poutP[:, I, :], xq[:])
                nc.vector.tensor_add(
                    outrow[:, I * P : (I + 1) * P],
                    pm[:],
                    en[:, I * P : (I + 1) * P],
                )
            nc.gpsimd.dma_start(out=out_d[b], in_=outrow[:])
            if debug_out and b == 0:
                nc.gpsimd.dma_start(out=dbg_acc_d[:], in_=acc_sb[:])
                ppsb = orp.tile([DIM, NCHUNK, P], f32, tag="dbgpp")
                nc.vector.tensor_copy(ppsb[:], poutP[:])
                nc.gpsimd.dma_start(out=dbg_pp_d[:], in_=ppsb[:])
                pqsb = orp.tile([DIM, NCHUNK, P], f32, tag="dbgpq")
                nc.vector.tensor_copy(pqsb[:], poutQ[:])
                nc.gpsimd.dma_start(out=dbg_pq_d[:], in_=pqsb[:])

    _spread_sync_waits(nc)
    return nc


def _ensure_ntff_hook():
    """bass_utils' axon trace path imports antenv.axon_hooks, which the image's
    antenv package lacks. Register an equivalent module backed by the boot
    package's ctypes NTFF hook so trace=True works; degrade silently if the
    pieces are missing (tracing is optional)."""
    import os
    import types

    try:
        import antenv.axon_hooks  # noqa: F401

        return
    except ImportError:
        pass
    try:
        import antenv
    except ImportError:
        return
    mod = types.ModuleType("antenv.axon_hooks")
    box = {"h": None}
    mod.set_axon_ntff_profile_hook = lambda h: box.__setitem__("h", h)
    mod.get_axon_ntff_profile_hook = lambda: box["h"]
    sys.modules["antenv.axon_hooks"] = mod
    antenv.axon_hooks = mod
    try:
        from trn_agent_boot.trn_boot import _ntff_profile_via_ctypes

        so = "/opt/axon/libaxon_pjrt.so"
        if os.path.exists(so):
            hook = _ntff_profile_via_ctypes(so)
            if hook is not None:
                mod.set_axon_ntff_profile_hook(hook)
    except Exception:
        pass


def kernel(x, eta_w1, eta_b1, eta_w2, eta_b2, mu_w1, mu_b1, mu_w2, mu_b2):
    global LAST_RESULT
    _ensure_ntff_hook()
    from concourse.bass_utils import run_bass_kernel_spmd

    f32 = np.float32
    x = np.ascontiguousarray(np.asarray(x, dtype=f32))
    eta_w1 = np.asarray(eta_w1, f32)
    eta_b1 = np.asarray(eta_b1, f32)
    eta_w2 = np.asarray(eta_w2, f32)
    eta_b2 = np.asarray(eta_b2, f32)
    mu_w1 = np.asarray(mu_w1, f32)
    mu_b1 = np.asarray(mu_b1, f32)
    mu_w2 = np.asarray(mu_w2, f32)
    mu_b2 = np.asarray(mu_b2, f32)

    nc = _build_program(float(-eta_b2[0]), float(mu_b2[0]))

    w2diag = np.zeros((P, H, P), f32)
    idx = np.arange(P)
    w2diag[idx, :, idx] = -eta_w2[:, 0][None, :]
    etas = np.zeros((P, 2, H), f32)
    etas[:, 0, :] = eta_w1[0][None, :]
    etas[:, 1, :] = eta_b1[None, :]
    mus = np.stack([mu_w1[0], mu_b1], axis=1).astype(f32)  # [H, 2]
    muw2 = np.repeat(mu_w2, DIM, axis=1).astype(f32)  # [H, DIM]
    ident = np.eye(P, dtype=f32)

    in_maps = []
    for core in range(NCORES):
        xc = np.ascontiguousarray(x[core * BPC : (core + 1) * BPC])
        xTc = xc.transpose(0, 2, 1)  # [BPC, DIM, N]
        n2 = (xc ** 2).sum(axis=2)  # [BPC, N]
        xTn = np.concatenate(
            [xTc, n2[:, None, :]], axis=1
        ).transpose(1, 0, 2)  # [DIM+1, BPC, N]
        statd = np.empty((DIM + 1, BPC, NCHUNK, P), f32)
        xin2 = np.empty((P, BPC, NCHUNK), f32)
        for bb in range(BPC):
            for I in range(NCHUNK):
                statd[0:DIM, bb, I, :] = -2.0 * xTc[bb, :, I * P : (I + 1) * P]
                statd[DIM, bb, I, :] = 1.0
                xin2[:, bb, I] = n2[bb, I * P : (I + 1) * P]
        in_maps.append(
            {
                "x": xc,
                "xTn": np.ascontiguousarray(xTn),
                "statd": statd,
                "xin2": xin2,
                "w2diag": w2diag,
                "etas": etas,
                "mus": mus,
                "muw2": muw2,
                "ident": ident,
            }
        )

    res = run_bass_kernel_spmd(nc, in_maps, core_ids=list(range(NCORES)))
    LAST_RESULT = res
    out = np.concatenate([r["out"] for r in res.results], axis=0)  # [B, DIM, N]
    return np.ascontiguousarray(out.transpose(0, 2, 1)).astype(np.float32)
